# revision 15
# baseline (speedup 1.0000x reference)
"""DeepSets segment-reduce kernel for 8x Trainium2 NeuronCores.

Strategy (all shapes hardcoded for N=500000, C=H=128, O=64, NSEG=2048):
  - Transposed activation layout: features on SBUF partitions, nodes on the
    free axis, so segment reductions are free-axis DVE reduces.
  - Whole-segment sharding: every segment is assigned entirely to one core,
    round-robin by global sorted-width rank.  All 8 cores share an identical
    compile-time slot/tile geometry (SPMD-safe).  No collective is needed -
    the host gather is the unshard.
  - Encoder BN is folded into the linear weights; the whole encoder path is
    bf16 (inputs, weights, activations) so the PE gets fast weight loads,
    input DMA halves, and SBUF pressure drops.  PSUM stays fp32.
  - Pad columns DUPLICATE the slot's first real column.  The segment max is
    then exact on device; the known dup contribution to the segment sum is
    subtracted on the host (the host replays the bf16 encoder for each
    segment's first node, bit-matching the device values to ~1ulp).
  - Wide tiles: slots are packed D-at-a-time into up-to-1024-column tiles of
    uniform padded width, so each relu / pairwise-TT / reduce is a single
    instruction over a two-PSUM-bank access pattern - half the instruction
    and semaphore count of 512-wide tiles.  Matmuls split at the 512-column
    PSUM bank boundary.
  - Engine balance: ACT runs relu1+relu3 (+4/9 of relu2), DVE runs the rest
    of relu2, the 2x_1p bf16 pairwise pre-halving tensor_tensors, and the
    1x-locked final reduces on the halved inputs.
  - Software pipelining: tile t's p3 consumers are deferred into tile t+1
    and split by engine: relu3(t) is issued right after mm1(t+1) so it plugs
    the ACT queue while mm1 runs, and the reduces of t are issued after
    mm3(t+1) so they never head-of-line-block a ready relu1 on either
    strict-FIFO queue.
  - Final projection out = [sum|max|mean] @ Wo'.T + bo' runs per core on its
    own 256 segments; mean is handled by projecting sums through the mean
    block of Wo' and row-scaling by 1/count.
"""

import os
import sys

import numpy as np

if "/opt/trn_rl_repo" not in sys.path:
    sys.path.insert(0, "/opt/trn_rl_repo")

import concourse.bacc as bacc
import concourse.mybir as mybir
import concourse.tile as tile
from concourse import bass_utils

EPS = 1e-5
NSEG = 2048
NCORES = 8
C = 128
H = 128
O = 64
S = NSEG // NCORES  # segment slots per core (256)
WIDE = 1024  # two PSUM banks of fp32
# Idempotent LDWEIGHTS padding per matmul pair: keeps the PE array active so
# the HAM clock gate holds K=8/8 (2.4 GHz) instead of oscillating to 1.2 GHz
# during the per-tile PE idle gaps.  Each dummy load streams w1s through the
# array (~107 ns busy, no PSUM write) and is overwritten by the next real
# matmul's own self-loading weights.
LDW_PAD = int(os.environ.get("KERNEL_LDW_PAD", "0"))

_compiled_cache = {}


def _fold_bn(W, b, g, be, m, v):
    a = g / np.sqrt(v + EPS)
    Wp = W * a[:, None]
    bp = (b - m) * a + be
    return Wp.astype(np.float32), bp.astype(np.float32)


def _plan_tiles(slot_w):
    """Pack slots (widths descending) into <=WIDE-column tiles of uniform
    padded width (multiple of 4 so bf16 half-offsets stay 4B-aligned for the
    DVE 2x_1p mode).  Returns (slot_start, n_slots, width, col_start) tiles
    plus total columns."""
    tiles = []
    col = 0
    k = 0
    n = len(slot_w)
    while k < n:
        wt = (int(slot_w[k]) + 3) & ~3
        assert 0 < wt <= WIDE // 2, f"slot width {wt} unsupported"
        d = min(WIDE // wt, n - k)
        tiles.append((k, d, wt, col))
        col += d * wt
        k += d
    return tiles, col


def _build_program(tiles, cols):
    """Emit the Bass/Tile program shared by all 8 cores."""
    nc = bacc.Bacc(
        "TRN2",
        target_bir_lowering=False,
        debug=False,
        num_devices=NCORES,
    )
    f32 = mybir.dt.float32
    bf16 = mybir.dt.bfloat16

    xT = nc.dram_tensor("xT", [C, cols], bf16, kind="ExternalInput").ap()
    w1 = nc.dram_tensor("w1", [C, H], bf16, kind="ExternalInput").ap()
    w2 = nc.dram_tensor("w2", [H, H], bf16, kind="ExternalInput").ap()
    w3 = nc.dram_tensor("w3", [H, H], bf16, kind="ExternalInput").ap()
    b1 = nc.dram_tensor("b1", [H, 1], f32, kind="ExternalInput").ap()
    b2 = nc.dram_tensor("b2", [H, 1], f32, kind="ExternalInput").ap()
    b3 = nc.dram_tensor("b3", [H, 1], f32, kind="ExternalInput").ap()
    wsum = nc.dram_tensor("wsum", [H, O], f32, kind="ExternalInput").ap()
    wmax = nc.dram_tensor("wmax", [H, O], f32, kind="ExternalInput").ap()
    wmean = nc.dram_tensor("wmean", [H, O], f32, kind="ExternalInput").ap()
    bo = nc.dram_tensor("bo", [1, O], f32, kind="ExternalInput").ap()
    # column ch holds the reciprocals for segment chunk ch (128 slots each)
    recip = nc.dram_tensor("recip", [H, S // H], f32, kind="ExternalInput").ap()
    out = nc.dram_tensor("out", [S, O], f32, kind="ExternalOutput").ap()

    relu = mybir.ActivationFunctionType.Relu
    add = mybir.AluOpType.add
    amax = mybir.AluOpType.max

    with tile.TileContext(nc) as tc:
        with (
            tc.tile_pool(name="const", bufs=1) as cpool,
            tc.tile_pool(name="xin", bufs=6) as xpool,
            tc.tile_pool(name="h1", bufs=2) as h1pool,
            tc.tile_pool(name="h2", bufs=2) as h2pool,
            tc.tile_pool(name="h3", bufs=3) as h3pool,
            tc.tile_pool(name="hm", bufs=3) as hmpool,
            tc.tile_pool(name="ht", bufs=3) as htpool,
            tc.tile_pool(name="acc", bufs=1) as accpool,
            tc.tile_pool(name="ps1", bufs=1, space="PSUM") as ps1,
            tc.tile_pool(name="ps2", bufs=1, space="PSUM") as ps2,
            tc.tile_pool(name="ps3", bufs=2, space="PSUM") as ps3,
        ):
            w1s = cpool.tile([C, H], bf16, tag="w1")
            w2s = cpool.tile([H, H], bf16, tag="w2")
            w3s = cpool.tile([H, H], bf16, tag="w3")
            b1s = cpool.tile([H, 1], f32, tag="b1")
            b2s = cpool.tile([H, 1], f32, tag="b2")
            b3s = cpool.tile([H, 1], f32, tag="b3")
            wsums = cpool.tile([H, O], f32, tag="wsum")
            wmaxs = cpool.tile([H, O], f32, tag="wmax")
            wmeans = cpool.tile([H, O], f32, tag="wmean")
            bos = cpool.tile([1, O], f32, tag="bo")
            recs = cpool.tile([H, S // H], f32, tag="recip")
            ones = cpool.tile([1, H], f32, tag="ones")

            nc.sync.dma_start(w1s[:], w1)
            nc.sync.dma_start(w2s[:], w2)
            nc.sync.dma_start(w3s[:], w3)
            nc.sync.dma_start(b1s[:], b1)
            nc.sync.dma_start(b2s[:], b2)
            nc.sync.dma_start(b3s[:], b3)
            nc.sync.dma_start(wsums[:], wsum)
            nc.sync.dma_start(wmaxs[:], wmax)
            nc.sync.dma_start(wmeans[:], wmean)
            nc.sync.dma_start(bos[:], bo)
            nc.sync.dma_start(recs[:], recip)
            nc.vector.memset(ones[:], 1.0)

            # Persistent per-slot partials (both post-relu, bias included).
            sumP = accpool.tile([H, S], f32, tag="sumP")
            maxP = accpool.tile([H, S], f32, tag="maxP")

            def consume_p3_relu(p3w, h3w, k0, d, wt, tcols):
                # split at the PSUM bank boundary: two always-ready ACTIVATEs
                # keep the ACT queue fed across the scheduler's optimistic
                # placement of relu1a (which waits on a possibly-cold mm1).
                r3s = min(tcols, 512)
                nc.scalar.activation(
                    h3w[:, :r3s], p3w[:, :r3s], relu, bias=b3s[:]
                )
                if tcols > 512:
                    nc.scalar.activation(
                        h3w[:, 512:tcols], p3w[:, 512:tcols], relu, bias=b3s[:]
                    )

            def consume_p3_reduce(p3w, h3w, k0, d, wt, tcols):
                h3v = h3w[:, :tcols].rearrange("p (d w) -> p d w", d=d)
                hw = wt // 2
                # DVE pre-halves both reduce inputs with 2x_1p bf16
                # tensor_tensor, then the 1x-locked reduce sees half the
                # columns.
                hm = hmpool.tile([H, WIDE // 2], bf16, tag="hm")
                hmv = hm[:, : d * hw].rearrange("p (d w) -> p d w", d=d)
                nc.vector.tensor_tensor(
                    hmv, h3v[:, :, :hw], h3v[:, :, hw:wt], op=amax
                )
                nc.vector.reduce_max(
                    maxP[:, k0 : k0 + d], hmv, axis=mybir.AxisListType.X
                )
                ht = htpool.tile([H, WIDE // 2], bf16, tag="ht")
                htv = ht[:, : d * hw].rearrange("p (d w) -> p d w", d=d)
                nc.vector.tensor_tensor(
                    htv, h3v[:, :, :hw], h3v[:, :, hw:wt], op=add
                )
                nc.vector.reduce_sum(
                    sumP[:, k0 : k0 + d], htv, axis=mybir.AxisListType.X
                )

            prev = None
            for ti, (k0, d, wt, col0) in enumerate(tiles):
                tcols = d * wt
                xt = xpool.tile([C, WIDE], bf16, tag="xt")
                nc.sync.dma_start(xt[:, :tcols], xT[:, col0 : col0 + tcols])

                s0 = min(tcols, 512)
                p1 = ps1.tile([H, WIDE], f32, tag="p1")
                nc.tensor.matmul(p1[:, :s0], w1s[:], xt[:, :s0])
                if tcols > 512:
                    nc.tensor.matmul(p1[:, 512:tcols], w1s[:], xt[:, 512:tcols])
                for _ in range(LDW_PAD):
                    nc.tensor.ldweights(w1s[:, :4])
                if prev is not None:
                    consume_p3_relu(*prev)
                h1 = h1pool.tile([H, WIDE], bf16, tag="h1")
                # relu1 halves run on ACT and DVE in parallel so p1 (the
                # single-buffered stage gating mm1 of the next tile) frees
                # in one half-pass latency.
                nc.scalar.activation(h1[:, :s0], p1[:, :s0], relu, bias=b1s[:])
                if tcols > 512:
                    nc.vector.tensor_scalar(
                        h1[:, 512:tcols], p1[:, 512:tcols], b1s[:], 0.0,
                        op0=add, op1=amax,
                    )

                p2 = ps2.tile([H, WIDE], f32, tag="p2")
                nc.tensor.matmul(p2[:, :s0], w2s[:], h1[:, :s0])
                if tcols > 512:
                    nc.tensor.matmul(p2[:, 512:tcols], w2s[:], h1[:, 512:tcols])
                for _ in range(LDW_PAD):
                    nc.tensor.ldweights(w1s[:, :4])
                h2 = h2pool.tile([H, WIDE], bf16, tag="h2")
                nc.scalar.activation(h2[:, :tcols], p2[:, :tcols], relu, bias=b2s[:])

                p3 = ps3.tile([H, WIDE], f32, tag="p3")
                nc.tensor.matmul(p3[:, :s0], w3s[:], h2[:, :s0])
                if tcols > 512:
                    nc.tensor.matmul(p3[:, 512:tcols], w3s[:], h2[:, 512:tcols])

                # Deferred consumption of the PREVIOUS tile's p3 keeps the
                # ACT queue from head-of-line-blocking on the 3-matmul chain.
                if prev is not None:
                    consume_p3_reduce(*prev)
                h3 = h3pool.tile([H, WIDE], bf16, tag="h3")
                prev = (p3, h3, k0, d, wt, tcols)
            consume_p3_relu(*prev)
            consume_p3_reduce(*prev)

            # ---- epilogue: out[k, :] = sum_k @ Wsum + max_k @ Wmax
            #                + (sum_k * recip_k) @ Wmean + bo ----
            for ch in range(S // H):  # 2 chunks of 128 segments
                sl = slice(ch * H, (ch + 1) * H)
                pow_ = ps1.tile([H, WIDE], f32, tag="p1")
                po = pow_[:, :O]
                nc.tensor.matmul(po[:], sumP[:, sl], wsums[:], start=True, stop=False)
                nc.tensor.matmul(po[:], maxP[:, sl], wmaxs[:], start=False, stop=False)
                nc.tensor.matmul(po[:], ones[:], bos[:], start=False, stop=True)

                pmw = ps2.tile([H, WIDE], f32, tag="p2")
                pm = pmw[:, :O]
                nc.tensor.matmul(pm[:], sumP[:, sl], wmeans[:], start=True, stop=True)

                om = h1pool.tile([H, O], f32, tag="om")
                nc.vector.tensor_scalar_mul(om[:], pm[:], recs[:, ch : ch + 1])
                ot = h2pool.tile([H, O], f32, tag="ot")
                nc.vector.tensor_tensor(ot[:], po[:], om[:], op=add)
                nc.sync.dma_start(out[sl, :], ot[:])

    nc.compile()
    return nc


def kernel(**inputs):
    x = np.asarray(inputs["x"], dtype=np.float32)
    batch = np.asarray(inputs["batch"]).astype(np.int64)

    # ---- fold BN into the linears ----
    W1p, b1p = _fold_bn(
        np.asarray(inputs["W1"]), np.asarray(inputs["b1"]),
        np.asarray(inputs["g1"]), np.asarray(inputs["be1"]),
        np.asarray(inputs["m1"]), np.asarray(inputs["v1"]),
    )
    W2p, b2p = _fold_bn(
        np.asarray(inputs["W2"]), np.asarray(inputs["b2"]),
        np.asarray(inputs["g2"]), np.asarray(inputs["be2"]),
        np.asarray(inputs["m2"]), np.asarray(inputs["v2"]),
    )
    W3p, b3p = _fold_bn(
        np.asarray(inputs["W3"]), np.asarray(inputs["b3"]),
        np.asarray(inputs["g3"]), np.asarray(inputs["be3"]),
        np.asarray(inputs["m3"]), np.asarray(inputs["v3"]),
    )
    Wop, bop = _fold_bn(
        np.asarray(inputs["Wo"]), np.asarray(inputs["bo"]),
        np.asarray(inputs["go"]), np.asarray(inputs["beo"]),
        np.asarray(inputs["mo"]), np.asarray(inputs["vo"]),
    )

    # ---- whole-segment sharding by sorted-width round-robin rank ----
    counts = np.bincount(batch, minlength=NSEG).astype(np.int64)
    assert np.all(batch[:-1] <= batch[1:]), "batch must be sorted"
    order = np.argsort(-counts, kind="stable")  # segment ids, width desc
    slot_w = np.maximum(counts[order[::NCORES][:S]], 1)  # width of rank 8k
    tiles, cols = _plan_tiles(slot_w)

    key = (cols, tuple(slot_w.tolist()), LDW_PAD)
    if key not in _compiled_cache:
        _compiled_cache[key] = _build_program(tiles, cols)
    nc = _compiled_cache[key]

    # column start and padded width of each slot
    slot_col = np.zeros(S, dtype=np.int64)
    slot_wt = np.zeros(S, dtype=np.int64)
    for k0, d, wt, col0 in tiles:
        for j in range(d):
            slot_col[k0 + j] = col0 + j * wt
            slot_wt[k0 + j] = wt

    starts = np.searchsorted(batch, np.arange(NSEG), side="left")

    bf = mybir.dt.np(mybir.dt.bfloat16)
    W1bf = W1p.astype(bf).astype(np.float32)
    W2bf = W2p.astype(bf).astype(np.float32)
    W3bf = W3p.astype(bf).astype(np.float32)

    in_maps = []
    core_segs = []
    for c in range(NCORES):
        segs = order[np.arange(S) * NCORES + c]  # this core's segment ids
        core_segs.append(segs)
        src = np.zeros(cols, dtype=np.int64)
        emptyc = np.zeros(cols, dtype=bool)
        for k in range(S):
            s = segs[k]
            cnt = int(counts[s])
            c0 = slot_col[k]
            wt = int(slot_wt[k])
            if cnt:
                src[c0 : c0 + cnt] = np.arange(starts[s], starts[s] + cnt)
                # dup-pad with the segment's first node
                src[c0 + cnt : c0 + wt] = starts[s]
            else:
                emptyc[c0 : c0 + wt] = True
        xTc = x[src].T.astype(bf)
        if emptyc.any():
            xTc[:, emptyc] = 0
        recipc = (1.0 / np.maximum(counts[segs], 1.0)).astype(np.float32)
        in_maps.append(
            dict(
                xT=np.ascontiguousarray(xTc),
                w1=np.ascontiguousarray(W1p.T).astype(bf),
                w2=np.ascontiguousarray(W2p.T).astype(bf),
                w3=np.ascontiguousarray(W3p.T).astype(bf),
                b1=np.ascontiguousarray(b1p[:, None]),
                b2=np.ascontiguousarray(b2p[:, None]),
                b3=np.ascontiguousarray(b3p[:, None]),
                wsum=np.ascontiguousarray(Wop[:, 0:H].T),
                wmax=np.ascontiguousarray(Wop[:, H : 2 * H].T),
                wmean=np.ascontiguousarray(Wop[:, 2 * H : 3 * H].T),
                bo=np.ascontiguousarray(bop[None, :]),
                recip=np.ascontiguousarray(recipc.reshape(S // H, H).T),
            )
        )

    ncores_run = int(os.environ.get("KERNEL_NCORES", str(NCORES)))
    res = bass_utils.run_bass_kernel_spmd(
        nc,
        in_maps[:ncores_run],
        core_ids=list(range(ncores_run)),
        trace=bool(int(os.environ.get("KERNEL_TRACE", "0"))),
        tmpdir=os.environ.get("KERNEL_TRACE_DIR") or None,
    )
    kernel.last_results = res

    # ---- host-side dup-pad correction ----
    # Replay the bf16 encoder for every segment's first node, matching the
    # device values (bf16 weights/activations, fp32 accumulate) to ~1ulp.
    first = x[starts[order[: S * NCORES]].clip(0)]  # [S*NCORES, C] rank order
    xf = first.astype(bf).astype(np.float32)
    h1f = np.maximum(xf @ W1bf.T + b1p, 0.0).astype(bf).astype(np.float32)
    h2f = np.maximum(h1f @ W2bf.T + b2p, 0.0).astype(bf).astype(np.float32)
    h3f = np.maximum(h2f @ W3bf.T + b3p, 0.0).astype(bf).astype(np.float32)

    out_full = np.zeros((NSEG, O), dtype=np.float32)
    ranks = np.arange(S)
    for c in range(ncores_run):
        segs = core_segs[c]
        o = np.array(res.results[c]["out"], dtype=np.float32)
        npads = (slot_wt - counts[segs]).astype(np.float32)
        h3c = h3f[ranks * NCORES + c]  # [S, H] first-node h3 per slot
        recipc = 1.0 / np.maximum(counts[segs], 1.0)
        corr = (h3c @ Wop[:, 0:H].T) * npads[:, None]
        corr += (h3c @ Wop[:, 2 * H : 3 * H].T) * (npads * recipc)[:, None]
        o -= corr
        empty = counts[segs] == 0
        if empty.any():
            o[empty] = bop[None, :]
        out_full[segs] = o
    return out_full



# revision 16
# speedup vs baseline: 1.1017x; 1.1017x over previous
"""DeepSets segment-reduce kernel for 8x Trainium2 NeuronCores.

Strategy (all shapes hardcoded for N=500000, C=H=128, O=64, NSEG=2048):
  - Transposed activation layout: features on SBUF partitions, nodes on the
    free axis, so segment reductions are free-axis DVE reduces.
  - Whole-segment sharding: every segment is assigned entirely to one core,
    round-robin by global sorted-width rank.  All 8 cores share an identical
    compile-time slot/tile geometry (SPMD-safe).  No collective is needed -
    the host gather is the unshard.
  - Encoder BN is folded into the linear weights; the whole encoder path is
    bf16 (inputs, weights, activations) so the PE gets fast weight loads,
    input DMA halves, and SBUF pressure drops.  PSUM stays fp32.
  - Pad columns DUPLICATE the slot's first real column.  The segment max is
    then exact on device; the known dup contribution to the segment sum is
    subtracted on the host (the host replays the bf16 encoder for each
    segment's first node, bit-matching the device values to ~1ulp).
  - Wide tiles: slots are packed D-at-a-time into up-to-1024-column tiles of
    uniform padded width, so each relu / pairwise-TT / reduce is a single
    instruction over a two-PSUM-bank access pattern - half the instruction
    and semaphore count of 512-wide tiles.  Matmuls split at the 512-column
    PSUM bank boundary.
  - Engine balance: ACT runs relu1+relu3 (+4/9 of relu2), DVE runs the rest
    of relu2, the 2x_1p bf16 pairwise pre-halving tensor_tensors, and the
    1x-locked final reduces on the halved inputs.
  - Software pipelining: tile t's p3 consumers are deferred into tile t+1
    and split by engine: relu3(t) is issued right after mm1(t+1) so it plugs
    the ACT queue while mm1 runs, and the reduces of t are issued after
    mm3(t+1) so they never head-of-line-block a ready relu1 on either
    strict-FIFO queue.
  - Final projection out = [sum|max|mean] @ Wo'.T + bo' runs per core on its
    own 256 segments; mean is handled by projecting sums through the mean
    block of Wo' and row-scaling by 1/count.
"""

import os
import sys

import numpy as np

if "/opt/trn_rl_repo" not in sys.path:
    sys.path.insert(0, "/opt/trn_rl_repo")

import concourse.bacc as bacc
import concourse.mybir as mybir
import concourse.tile as tile
from concourse import bass_utils

EPS = 1e-5
NSEG = 2048
NCORES = 8
C = 128
H = 128
O = 64
S = NSEG // NCORES  # segment slots per core (256)
WIDE = 1024  # two PSUM banks of fp32
# Idempotent LDWEIGHTS padding per matmul pair: keeps the PE array active so
# the HAM clock gate holds K=8/8 (2.4 GHz) instead of oscillating to 1.2 GHz
# during the per-tile PE idle gaps.  Each dummy load streams w1s through the
# array (~107 ns busy, no PSUM write) and is overwritten by the next real
# matmul's own self-loading weights.
LDW_PAD = int(os.environ.get("KERNEL_LDW_PAD", "0"))

_compiled_cache = {}


def _fold_bn(W, b, g, be, m, v):
    a = g / np.sqrt(v + EPS)
    Wp = W * a[:, None]
    bp = (b - m) * a + be
    return Wp.astype(np.float32), bp.astype(np.float32)


def _plan_tiles(slot_w):
    """Pack slots (widths descending) into <=WIDE-column tiles of uniform
    padded width (multiple of 4 so bf16 half-offsets stay 4B-aligned for the
    DVE 2x_1p mode).  Returns (slot_start, n_slots, width, col_start) tiles
    plus total columns."""
    tiles = []
    col = 0
    k = 0
    n = len(slot_w)
    while k < n:
        wt = (int(slot_w[k]) + 3) & ~3
        assert 0 < wt <= WIDE // 2, f"slot width {wt} unsupported"
        d = min(WIDE // wt, n - k)
        tiles.append((k, d, wt, col))
        col += d * wt
        k += d
    return tiles, col


def _build_program(tiles, cols):
    """Emit the Bass/Tile program shared by all 8 cores."""
    nc = bacc.Bacc(
        "TRN2",
        target_bir_lowering=False,
        debug=False,
        num_devices=NCORES,
    )
    f32 = mybir.dt.float32
    bf16 = mybir.dt.bfloat16

    xT = nc.dram_tensor("xT", [C, cols], bf16, kind="ExternalInput").ap()
    w1 = nc.dram_tensor("w1", [C, H], bf16, kind="ExternalInput").ap()
    w2 = nc.dram_tensor("w2", [H, H], bf16, kind="ExternalInput").ap()
    w3 = nc.dram_tensor("w3", [H, H], bf16, kind="ExternalInput").ap()
    b1 = nc.dram_tensor("b1", [H, 1], f32, kind="ExternalInput").ap()
    b2 = nc.dram_tensor("b2", [H, 1], f32, kind="ExternalInput").ap()
    b3 = nc.dram_tensor("b3", [H, 1], f32, kind="ExternalInput").ap()
    wsum = nc.dram_tensor("wsum", [H, O], f32, kind="ExternalInput").ap()
    wmax = nc.dram_tensor("wmax", [H, O], f32, kind="ExternalInput").ap()
    wmean = nc.dram_tensor("wmean", [H, O], f32, kind="ExternalInput").ap()
    bo = nc.dram_tensor("bo", [1, O], f32, kind="ExternalInput").ap()
    # column ch holds the reciprocals for segment chunk ch (128 slots each)
    recip = nc.dram_tensor("recip", [H, S // H], f32, kind="ExternalInput").ap()
    out = nc.dram_tensor("out", [S, O], f32, kind="ExternalOutput").ap()

    relu = mybir.ActivationFunctionType.Relu
    add = mybir.AluOpType.add
    amax = mybir.AluOpType.max

    with tile.TileContext(nc) as tc:
        with (
            tc.tile_pool(name="const", bufs=1) as cpool,
            tc.tile_pool(name="xin", bufs=6) as xpool,
            tc.tile_pool(name="h1", bufs=2) as h1pool,
            tc.tile_pool(name="h2", bufs=2) as h2pool,
            tc.tile_pool(name="h3", bufs=3) as h3pool,
            tc.tile_pool(name="hm", bufs=3) as hmpool,
            tc.tile_pool(name="ht", bufs=3) as htpool,
            tc.tile_pool(name="acc", bufs=1) as accpool,
            tc.tile_pool(name="ps1", bufs=1, space="PSUM") as ps1,
            tc.tile_pool(name="ps2", bufs=1, space="PSUM") as ps2,
            tc.tile_pool(name="ps3", bufs=2, space="PSUM") as ps3,
        ):
            w1s = cpool.tile([C, H], bf16, tag="w1")
            w2s = cpool.tile([H, H], bf16, tag="w2")
            w3s = cpool.tile([H, H], bf16, tag="w3")
            b1s = cpool.tile([H, 1], f32, tag="b1")
            b2s = cpool.tile([H, 1], f32, tag="b2")
            b3s = cpool.tile([H, 1], f32, tag="b3")
            wsums = cpool.tile([H, O], f32, tag="wsum")
            wmaxs = cpool.tile([H, O], f32, tag="wmax")
            wmeans = cpool.tile([H, O], f32, tag="wmean")
            bos = cpool.tile([1, O], f32, tag="bo")
            recs = cpool.tile([H, S // H], f32, tag="recip")
            ones = cpool.tile([1, H], f32, tag="ones")

            nc.sync.dma_start(w1s[:], w1)
            nc.sync.dma_start(w2s[:], w2)
            nc.sync.dma_start(w3s[:], w3)
            nc.sync.dma_start(b1s[:], b1)
            nc.sync.dma_start(b2s[:], b2)
            nc.sync.dma_start(b3s[:], b3)
            nc.sync.dma_start(wsums[:], wsum)
            nc.sync.dma_start(wmaxs[:], wmax)
            nc.sync.dma_start(wmeans[:], wmean)
            nc.sync.dma_start(bos[:], bo)
            nc.sync.dma_start(recs[:], recip)
            nc.vector.memset(ones[:], 1.0)

            # Persistent per-slot partials (both post-relu, bias included).
            sumP = accpool.tile([H, S], f32, tag="sumP")
            maxP = accpool.tile([H, S], f32, tag="maxP")

            def consume_p3_relu(p3w, h3w, k0, d, wt, tcols):
                nc.scalar.activation(
                    h3w[:, :tcols], p3w[:, :tcols], relu, bias=b3s[:]
                )

            def consume_p3_reduce(p3w, h3w, k0, d, wt, tcols):
                h3v = h3w[:, :tcols].rearrange("p (d w) -> p d w", d=d)
                hw = wt // 2
                # DVE pre-halves both reduce inputs with 2x_1p bf16
                # tensor_tensor, then the 1x-locked reduce sees half the
                # columns.
                hm = hmpool.tile([H, WIDE // 2], bf16, tag="hm")
                hmv = hm[:, : d * hw].rearrange("p (d w) -> p d w", d=d)
                nc.vector.tensor_tensor(
                    hmv, h3v[:, :, :hw], h3v[:, :, hw:wt], op=amax
                )
                nc.vector.reduce_max(
                    maxP[:, k0 : k0 + d], hmv, axis=mybir.AxisListType.X
                )
                ht = htpool.tile([H, WIDE // 2], bf16, tag="ht")
                htv = ht[:, : d * hw].rearrange("p (d w) -> p d w", d=d)
                nc.vector.tensor_tensor(
                    htv, h3v[:, :, :hw], h3v[:, :, hw:wt], op=add
                )
                nc.vector.reduce_sum(
                    sumP[:, k0 : k0 + d], htv, axis=mybir.AxisListType.X
                )

            prev = None
            for ti, (k0, d, wt, col0) in enumerate(tiles):
                tcols = d * wt
                xt = xpool.tile([C, WIDE], bf16, tag="xt")
                nc.sync.dma_start(xt[:, :tcols], xT[:, col0 : col0 + tcols])

                s0 = min(tcols, 512)
                p1 = ps1.tile([H, WIDE], f32, tag="p1")
                nc.tensor.matmul(p1[:, :s0], w1s[:], xt[:, :s0])
                if tcols > 512:
                    nc.tensor.matmul(p1[:, 512:tcols], w1s[:], xt[:, 512:tcols])
                for _ in range(LDW_PAD):
                    nc.tensor.ldweights(w1s[:, :4])
                if prev is not None:
                    consume_p3_relu(*prev)
                h1 = h1pool.tile([H, WIDE], bf16, tag="h1")
                # relu1 halves run on ACT and DVE in parallel so p1 (the
                # single-buffered stage gating mm1 of the next tile) frees
                # in one half-pass latency.
                nc.scalar.activation(h1[:, :s0], p1[:, :s0], relu, bias=b1s[:])
                if tcols > 512:
                    nc.vector.tensor_scalar(
                        h1[:, 512:tcols], p1[:, 512:tcols], b1s[:], 0.0,
                        op0=add, op1=amax,
                    )

                p2 = ps2.tile([H, WIDE], f32, tag="p2")
                nc.tensor.matmul(p2[:, :s0], w2s[:], h1[:, :s0])
                if tcols > 512:
                    nc.tensor.matmul(p2[:, 512:tcols], w2s[:], h1[:, 512:tcols])
                for _ in range(LDW_PAD):
                    nc.tensor.ldweights(w1s[:, :4])
                h2 = h2pool.tile([H, WIDE], bf16, tag="h2")
                nc.scalar.activation(h2[:, :tcols], p2[:, :tcols], relu, bias=b2s[:])

                p3 = ps3.tile([H, WIDE], f32, tag="p3")
                nc.tensor.matmul(p3[:, :s0], w3s[:], h2[:, :s0])
                if tcols > 512:
                    nc.tensor.matmul(p3[:, 512:tcols], w3s[:], h2[:, 512:tcols])

                # Deferred consumption of the PREVIOUS tile's p3 keeps the
                # ACT queue from head-of-line-blocking on the 3-matmul chain.
                if prev is not None:
                    consume_p3_reduce(*prev)
                h3 = h3pool.tile([H, WIDE], bf16, tag="h3")
                prev = (p3, h3, k0, d, wt, tcols)
            consume_p3_relu(*prev)
            consume_p3_reduce(*prev)

            # ---- epilogue: out[k, :] = sum_k @ Wsum + max_k @ Wmax
            #                + (sum_k * recip_k) @ Wmean + bo ----
            for ch in range(S // H):  # 2 chunks of 128 segments
                sl = slice(ch * H, (ch + 1) * H)
                pow_ = ps1.tile([H, WIDE], f32, tag="p1")
                po = pow_[:, :O]
                nc.tensor.matmul(po[:], sumP[:, sl], wsums[:], start=True, stop=False)
                nc.tensor.matmul(po[:], maxP[:, sl], wmaxs[:], start=False, stop=False)
                nc.tensor.matmul(po[:], ones[:], bos[:], start=False, stop=True)

                pmw = ps2.tile([H, WIDE], f32, tag="p2")
                pm = pmw[:, :O]
                nc.tensor.matmul(pm[:], sumP[:, sl], wmeans[:], start=True, stop=True)

                om = h1pool.tile([H, O], f32, tag="om")
                nc.vector.tensor_scalar_mul(om[:], pm[:], recs[:, ch : ch + 1])
                ot = h2pool.tile([H, O], f32, tag="ot")
                nc.vector.tensor_tensor(ot[:], po[:], om[:], op=add)
                nc.sync.dma_start(out[sl, :], ot[:])

    nc.compile()
    return nc


def kernel(**inputs):
    x = np.asarray(inputs["x"], dtype=np.float32)
    batch = np.asarray(inputs["batch"]).astype(np.int64)

    # ---- fold BN into the linears ----
    W1p, b1p = _fold_bn(
        np.asarray(inputs["W1"]), np.asarray(inputs["b1"]),
        np.asarray(inputs["g1"]), np.asarray(inputs["be1"]),
        np.asarray(inputs["m1"]), np.asarray(inputs["v1"]),
    )
    W2p, b2p = _fold_bn(
        np.asarray(inputs["W2"]), np.asarray(inputs["b2"]),
        np.asarray(inputs["g2"]), np.asarray(inputs["be2"]),
        np.asarray(inputs["m2"]), np.asarray(inputs["v2"]),
    )
    W3p, b3p = _fold_bn(
        np.asarray(inputs["W3"]), np.asarray(inputs["b3"]),
        np.asarray(inputs["g3"]), np.asarray(inputs["be3"]),
        np.asarray(inputs["m3"]), np.asarray(inputs["v3"]),
    )
    Wop, bop = _fold_bn(
        np.asarray(inputs["Wo"]), np.asarray(inputs["bo"]),
        np.asarray(inputs["go"]), np.asarray(inputs["beo"]),
        np.asarray(inputs["mo"]), np.asarray(inputs["vo"]),
    )

    # ---- whole-segment sharding by sorted-width round-robin rank ----
    counts = np.bincount(batch, minlength=NSEG).astype(np.int64)
    assert np.all(batch[:-1] <= batch[1:]), "batch must be sorted"
    order = np.argsort(-counts, kind="stable")  # segment ids, width desc
    slot_w = np.maximum(counts[order[::NCORES][:S]], 1)  # width of rank 8k
    tiles, cols = _plan_tiles(slot_w)

    key = (cols, tuple(slot_w.tolist()), LDW_PAD)
    if key not in _compiled_cache:
        _compiled_cache[key] = _build_program(tiles, cols)
    nc = _compiled_cache[key]

    # column start and padded width of each slot
    slot_col = np.zeros(S, dtype=np.int64)
    slot_wt = np.zeros(S, dtype=np.int64)
    for k0, d, wt, col0 in tiles:
        for j in range(d):
            slot_col[k0 + j] = col0 + j * wt
            slot_wt[k0 + j] = wt

    starts = np.searchsorted(batch, np.arange(NSEG), side="left")

    bf = mybir.dt.np(mybir.dt.bfloat16)
    W1bf = W1p.astype(bf).astype(np.float32)
    W2bf = W2p.astype(bf).astype(np.float32)
    W3bf = W3p.astype(bf).astype(np.float32)

    in_maps = []
    core_segs = []
    for c in range(NCORES):
        segs = order[np.arange(S) * NCORES + c]  # this core's segment ids
        core_segs.append(segs)
        src = np.zeros(cols, dtype=np.int64)
        emptyc = np.zeros(cols, dtype=bool)
        for k in range(S):
            s = segs[k]
            cnt = int(counts[s])
            c0 = slot_col[k]
            wt = int(slot_wt[k])
            if cnt:
                src[c0 : c0 + cnt] = np.arange(starts[s], starts[s] + cnt)
                # dup-pad with the segment's first node
                src[c0 + cnt : c0 + wt] = starts[s]
            else:
                emptyc[c0 : c0 + wt] = True
        xTc = x[src].T.astype(bf)
        if emptyc.any():
            xTc[:, emptyc] = 0
        recipc = (1.0 / np.maximum(counts[segs], 1.0)).astype(np.float32)
        in_maps.append(
            dict(
                xT=np.ascontiguousarray(xTc),
                w1=np.ascontiguousarray(W1p.T).astype(bf),
                w2=np.ascontiguousarray(W2p.T).astype(bf),
                w3=np.ascontiguousarray(W3p.T).astype(bf),
                b1=np.ascontiguousarray(b1p[:, None]),
                b2=np.ascontiguousarray(b2p[:, None]),
                b3=np.ascontiguousarray(b3p[:, None]),
                wsum=np.ascontiguousarray(Wop[:, 0:H].T),
                wmax=np.ascontiguousarray(Wop[:, H : 2 * H].T),
                wmean=np.ascontiguousarray(Wop[:, 2 * H : 3 * H].T),
                bo=np.ascontiguousarray(bop[None, :]),
                recip=np.ascontiguousarray(recipc.reshape(S // H, H).T),
            )
        )

    ncores_run = int(os.environ.get("KERNEL_NCORES", str(NCORES)))
    res = bass_utils.run_bass_kernel_spmd(
        nc,
        in_maps[:ncores_run],
        core_ids=list(range(ncores_run)),
        trace=bool(int(os.environ.get("KERNEL_TRACE", "0"))),
        tmpdir=os.environ.get("KERNEL_TRACE_DIR") or None,
    )
    kernel.last_results = res

    # ---- host-side dup-pad correction ----
    # Replay the bf16 encoder for every segment's first node, matching the
    # device values (bf16 weights/activations, fp32 accumulate) to ~1ulp.
    first = x[starts[order[: S * NCORES]].clip(0)]  # [S*NCORES, C] rank order
    xf = first.astype(bf).astype(np.float32)
    h1f = np.maximum(xf @ W1bf.T + b1p, 0.0).astype(bf).astype(np.float32)
    h2f = np.maximum(h1f @ W2bf.T + b2p, 0.0).astype(bf).astype(np.float32)
    h3f = np.maximum(h2f @ W3bf.T + b3p, 0.0).astype(bf).astype(np.float32)

    out_full = np.zeros((NSEG, O), dtype=np.float32)
    ranks = np.arange(S)
    for c in range(ncores_run):
        segs = core_segs[c]
        o = np.array(res.results[c]["out"], dtype=np.float32)
        npads = (slot_wt - counts[segs]).astype(np.float32)
        h3c = h3f[ranks * NCORES + c]  # [S, H] first-node h3 per slot
        recipc = 1.0 / np.maximum(counts[segs], 1.0)
        corr = (h3c @ Wop[:, 0:H].T) * npads[:, None]
        corr += (h3c @ Wop[:, 2 * H : 3 * H].T) * (npads * recipc)[:, None]
        o -= corr
        empty = counts[segs] == 0
        if empty.any():
            o[empty] = bop[None, :]
        out_full[segs] = o
    return out_full



# revision 18
# speedup vs baseline: 1.1039x; 1.0020x over previous
"""DeepSets segment-reduce kernel for 8x Trainium2 NeuronCores.

Strategy (all shapes hardcoded for N=500000, C=H=128, O=64, NSEG=2048):
  - Transposed activation layout: features on SBUF partitions, nodes on the
    free axis, so segment reductions are free-axis DVE reduces.
  - Whole-segment sharding: every segment is assigned entirely to one core,
    round-robin by global sorted-width rank.  All 8 cores share an identical
    compile-time slot/tile geometry (SPMD-safe).  No collective is needed -
    the host gather is the unshard.
  - Encoder BN is folded into the linear weights; the whole encoder path is
    bf16 (inputs, weights, activations) so the PE gets fast weight loads,
    input DMA halves, and SBUF pressure drops.  PSUM stays fp32.
  - Pad columns DUPLICATE the slot's first real column.  The segment max is
    then exact on device; the known dup contribution to the segment sum is
    subtracted on the host (the host replays the bf16 encoder for each
    segment's first node, bit-matching the device values to ~1ulp).
  - Wide tiles: slots are packed D-at-a-time into up-to-1024-column tiles of
    uniform padded width, so each relu / pairwise-TT / reduce is a single
    instruction over a two-PSUM-bank access pattern - half the instruction
    and semaphore count of 512-wide tiles.  Matmuls split at the 512-column
    PSUM bank boundary.
  - Engine balance: ACT runs relu1+relu3 (+4/9 of relu2), DVE runs the rest
    of relu2, the 2x_1p bf16 pairwise pre-halving tensor_tensors, and the
    1x-locked final reduces on the halved inputs.
  - Software pipelining: tile t's p3 consumers are deferred into tile t+1
    and split by engine: relu3(t) is issued right after mm1(t+1) so it plugs
    the ACT queue while mm1 runs, and the reduces of t are issued after
    mm3(t+1) so they never head-of-line-block a ready relu1 on either
    strict-FIFO queue.
  - Final projection out = [sum|max|mean] @ Wo'.T + bo' runs per core on its
    own 256 segments; mean is handled by projecting sums through the mean
    block of Wo' and row-scaling by 1/count.
"""

import os
import sys

import numpy as np

if "/opt/trn_rl_repo" not in sys.path:
    sys.path.insert(0, "/opt/trn_rl_repo")

import concourse.bacc as bacc
import concourse.mybir as mybir
import concourse.tile as tile
from concourse import bass_utils

EPS = 1e-5
NSEG = 2048
NCORES = 8
C = 128
H = 128
O = 64
S = NSEG // NCORES  # segment slots per core (256)
WIDE = 1024  # two PSUM banks of fp32
# Idempotent LDWEIGHTS padding per matmul pair: keeps the PE array active so
# the HAM clock gate holds K=8/8 (2.4 GHz) instead of oscillating to 1.2 GHz
# during the per-tile PE idle gaps.  Each dummy load streams w1s through the
# array (~107 ns busy, no PSUM write) and is overwritten by the next real
# matmul's own self-loading weights.
LDW_PAD = int(os.environ.get("KERNEL_LDW_PAD", "0"))
# relu1 ACT/DVE boundary; 512 = baseline single two-bank p1 tile.  Values in
# [402, 512) split p1 into two single-bank PSUM tiles so each engine reads
# within one bank (bank-crossing PSUM APs are slow).
R1B = int(os.environ.get("KERNEL_R1B", "512"))

_compiled_cache = {}


def _fold_bn(W, b, g, be, m, v):
    a = g / np.sqrt(v + EPS)
    Wp = W * a[:, None]
    bp = (b - m) * a + be
    return Wp.astype(np.float32), bp.astype(np.float32)


def _plan_tiles(slot_w):
    """Pack slots (widths descending) into <=WIDE-column tiles of uniform
    padded width (multiple of 4 so bf16 half-offsets stay 4B-aligned for the
    DVE 2x_1p mode).  Returns (slot_start, n_slots, width, col_start) tiles
    plus total columns."""
    tiles = []
    col = 0
    k = 0
    n = len(slot_w)
    while k < n:
        wt = (int(slot_w[k]) + 3) & ~3
        assert 0 < wt <= WIDE // 2, f"slot width {wt} unsupported"
        d = min(WIDE // wt, n - k)
        tiles.append((k, d, wt, col))
        col += d * wt
        k += d
    return tiles, col


def _build_program(tiles, cols):
    """Emit the Bass/Tile program shared by all 8 cores."""
    nc = bacc.Bacc(
        "TRN2",
        target_bir_lowering=False,
        debug=False,
        num_devices=NCORES,
    )
    f32 = mybir.dt.float32
    bf16 = mybir.dt.bfloat16

    xT = nc.dram_tensor("xT", [C, cols], bf16, kind="ExternalInput").ap()
    w1 = nc.dram_tensor("w1", [C, H], bf16, kind="ExternalInput").ap()
    w2 = nc.dram_tensor("w2", [H, H], bf16, kind="ExternalInput").ap()
    w3 = nc.dram_tensor("w3", [H, H], bf16, kind="ExternalInput").ap()
    b1 = nc.dram_tensor("b1", [H, 1], f32, kind="ExternalInput").ap()
    b2 = nc.dram_tensor("b2", [H, 1], f32, kind="ExternalInput").ap()
    b3 = nc.dram_tensor("b3", [H, 1], f32, kind="ExternalInput").ap()
    wsum = nc.dram_tensor("wsum", [H, O], f32, kind="ExternalInput").ap()
    wmax = nc.dram_tensor("wmax", [H, O], f32, kind="ExternalInput").ap()
    wmean = nc.dram_tensor("wmean", [H, O], f32, kind="ExternalInput").ap()
    bo = nc.dram_tensor("bo", [1, O], f32, kind="ExternalInput").ap()
    # column ch holds the reciprocals for segment chunk ch (128 slots each)
    recip = nc.dram_tensor("recip", [H, S // H], f32, kind="ExternalInput").ap()
    out = nc.dram_tensor("out", [S, O], f32, kind="ExternalOutput").ap()

    relu = mybir.ActivationFunctionType.Relu
    add = mybir.AluOpType.add
    amax = mybir.AluOpType.max

    with tile.TileContext(nc) as tc:
        with (
            tc.tile_pool(name="const", bufs=1) as cpool,
            tc.tile_pool(name="xin", bufs=6) as xpool,
            tc.tile_pool(name="h1", bufs=2) as h1pool,
            tc.tile_pool(name="h2", bufs=2) as h2pool,
            tc.tile_pool(name="h3", bufs=3) as h3pool,
            tc.tile_pool(name="hm", bufs=3) as hmpool,
            tc.tile_pool(name="ht", bufs=3) as htpool,
            tc.tile_pool(name="acc", bufs=1) as accpool,
            tc.tile_pool(name="ps1", bufs=1, space="PSUM") as ps1,
            tc.tile_pool(name="ps2", bufs=1, space="PSUM") as ps2,
            tc.tile_pool(name="ps3", bufs=2, space="PSUM") as ps3,
        ):
            w1s = cpool.tile([C, H], bf16, tag="w1")
            w2s = cpool.tile([H, H], bf16, tag="w2")
            w3s = cpool.tile([H, H], bf16, tag="w3")
            b1s = cpool.tile([H, 1], f32, tag="b1")
            b2s = cpool.tile([H, 1], f32, tag="b2")
            b3s = cpool.tile([H, 1], f32, tag="b3")
            wsums = cpool.tile([H, O], f32, tag="wsum")
            wmaxs = cpool.tile([H, O], f32, tag="wmax")
            wmeans = cpool.tile([H, O], f32, tag="wmean")
            bos = cpool.tile([1, O], f32, tag="bo")
            recs = cpool.tile([H, S // H], f32, tag="recip")
            ones = cpool.tile([1, H], f32, tag="ones")

            nc.sync.dma_start(w1s[:], w1)
            nc.sync.dma_start(w2s[:], w2)
            nc.sync.dma_start(w3s[:], w3)
            nc.sync.dma_start(b1s[:], b1)
            nc.sync.dma_start(b2s[:], b2)
            nc.sync.dma_start(b3s[:], b3)
            nc.sync.dma_start(wsums[:], wsum)
            nc.sync.dma_start(wmaxs[:], wmax)
            nc.sync.dma_start(wmeans[:], wmean)
            nc.sync.dma_start(bos[:], bo)
            nc.sync.dma_start(recs[:], recip)
            nc.vector.memset(ones[:], 1.0)

            # Persistent per-slot partials (both post-relu, bias included).
            sumP = accpool.tile([H, S], f32, tag="sumP")
            maxP = accpool.tile([H, S], f32, tag="maxP")

            def consume_p3_relu(p3w, h3w, k0, d, wt, tcols):
                nc.scalar.activation(
                    h3w[:, :tcols], p3w[:, :tcols], relu, bias=b3s[:]
                )

            def consume_p3_reduce(p3w, h3w, k0, d, wt, tcols):
                h3v = h3w[:, :tcols].rearrange("p (d w) -> p d w", d=d)
                hw = wt // 2
                # DVE pre-halves both reduce inputs with 2x_1p bf16
                # tensor_tensor, then the 1x-locked reduce sees half the
                # columns.
                hm = hmpool.tile([H, WIDE // 2], bf16, tag="hm")
                hmv = hm[:, : d * hw].rearrange("p (d w) -> p d w", d=d)
                nc.vector.tensor_tensor(
                    hmv, h3v[:, :, :hw], h3v[:, :, hw:wt], op=amax
                )
                nc.vector.reduce_max(
                    maxP[:, k0 : k0 + d], hmv, axis=mybir.AxisListType.X
                )
                ht = htpool.tile([H, WIDE // 2], bf16, tag="ht")
                htv = ht[:, : d * hw].rearrange("p (d w) -> p d w", d=d)
                nc.vector.tensor_tensor(
                    htv, h3v[:, :, :hw], h3v[:, :, hw:wt], op=add
                )
                nc.vector.reduce_sum(
                    sumP[:, k0 : k0 + d], htv, axis=mybir.AxisListType.X
                )

            prev = None
            for ti, (k0, d, wt, col0) in enumerate(tiles):
                tcols = d * wt
                xt = xpool.tile([C, WIDE], bf16, tag="xt")
                nc.sync.dma_start(xt[:, :tcols], xT[:, col0 : col0 + tcols])

                s0 = min(tcols, 512)
                split = R1B != 512 and tcols > 512
                b1x = max(min(R1B, s0), tcols - 512)
                if split:
                    p1aw = ps1.tile([H, 512], f32, tag="p1a")
                    p1bw = ps1.tile([H, 512], f32, tag="p1b")
                    p1a, p1b = p1aw[:, :b1x], p1bw[:, : tcols - b1x]
                    xa, xb = xt[:, :b1x], xt[:, b1x:tcols]
                else:
                    p1 = ps1.tile([H, WIDE], f32, tag="p1")
                    p1a = p1[:, :s0]
                    p1b = p1[:, 512:tcols] if tcols > 512 else None
                    xa = xt[:, :s0]
                    xb = xt[:, 512:tcols] if tcols > 512 else None
                nc.tensor.matmul(p1a, w1s[:], xa)
                if p1b is not None:
                    nc.tensor.matmul(p1b, w1s[:], xb)
                for _ in range(LDW_PAD):
                    nc.tensor.ldweights(w1s[:, :4])
                if prev is not None:
                    consume_p3_relu(*prev)
                h1 = h1pool.tile([H, WIDE], bf16, tag="h1")
                # relu1 halves run on ACT and DVE in parallel so p1 (the
                # single-buffered stage gating mm1 of the next tile) frees
                # in one half-pass latency.
                nc.scalar.activation(
                    h1[:, :b1x] if split else h1[:, :s0],
                    p1a, relu, bias=b1s[:],
                )
                if p1b is not None:
                    nc.vector.tensor_scalar(
                        h1[:, b1x:tcols] if split else h1[:, 512:tcols],
                        p1b, b1s[:], 0.0,
                        op0=add, op1=amax,
                    )

                p2 = ps2.tile([H, WIDE], f32, tag="p2")
                nc.tensor.matmul(p2[:, :s0], w2s[:], h1[:, :s0])
                if tcols > 512:
                    nc.tensor.matmul(p2[:, 512:tcols], w2s[:], h1[:, 512:tcols])
                for _ in range(LDW_PAD):
                    nc.tensor.ldweights(w1s[:, :4])
                h2 = h2pool.tile([H, WIDE], bf16, tag="h2")
                nc.scalar.activation(h2[:, :tcols], p2[:, :tcols], relu, bias=b2s[:])

                p3 = ps3.tile([H, WIDE], f32, tag="p3")
                nc.tensor.matmul(p3[:, :s0], w3s[:], h2[:, :s0])
                if tcols > 512:
                    nc.tensor.matmul(p3[:, 512:tcols], w3s[:], h2[:, 512:tcols])

                # Deferred consumption of the PREVIOUS tile's p3 keeps the
                # ACT queue from head-of-line-blocking on the 3-matmul chain.
                if prev is not None:
                    consume_p3_reduce(*prev)
                h3 = h3pool.tile([H, WIDE], bf16, tag="h3")
                prev = (p3, h3, k0, d, wt, tcols)
            consume_p3_relu(*prev)
            consume_p3_reduce(*prev)

            # ---- epilogue: out[k, :] = sum_k @ Wsum + max_k @ Wmax
            #                + (sum_k * recip_k) @ Wmean + bo ----
            for ch in range(S // H):  # 2 chunks of 128 segments
                sl = slice(ch * H, (ch + 1) * H)
                pow_ = ps1.tile([H, WIDE], f32, tag="p1")
                po = pow_[:, :O]
                nc.tensor.matmul(po[:], sumP[:, sl], wsums[:], start=True, stop=False)
                nc.tensor.matmul(po[:], maxP[:, sl], wmaxs[:], start=False, stop=False)
                nc.tensor.matmul(po[:], ones[:], bos[:], start=False, stop=True)

                pmw = ps2.tile([H, WIDE], f32, tag="p2")
                pm = pmw[:, :O]
                nc.tensor.matmul(pm[:], sumP[:, sl], wmeans[:], start=True, stop=True)

                om = h1pool.tile([H, O], f32, tag="om")
                nc.vector.tensor_scalar_mul(om[:], pm[:], recs[:, ch : ch + 1])
                ot = h2pool.tile([H, O], f32, tag="ot")
                nc.vector.tensor_tensor(ot[:], po[:], om[:], op=add)
                nc.sync.dma_start(out[sl, :], ot[:])

    nc.compile()
    return nc


def kernel(**inputs):
    x = np.asarray(inputs["x"], dtype=np.float32)
    batch = np.asarray(inputs["batch"]).astype(np.int64)

    # ---- fold BN into the linears ----
    W1p, b1p = _fold_bn(
        np.asarray(inputs["W1"]), np.asarray(inputs["b1"]),
        np.asarray(inputs["g1"]), np.asarray(inputs["be1"]),
        np.asarray(inputs["m1"]), np.asarray(inputs["v1"]),
    )
    W2p, b2p = _fold_bn(
        np.asarray(inputs["W2"]), np.asarray(inputs["b2"]),
        np.asarray(inputs["g2"]), np.asarray(inputs["be2"]),
        np.asarray(inputs["m2"]), np.asarray(inputs["v2"]),
    )
    W3p, b3p = _fold_bn(
        np.asarray(inputs["W3"]), np.asarray(inputs["b3"]),
        np.asarray(inputs["g3"]), np.asarray(inputs["be3"]),
        np.asarray(inputs["m3"]), np.asarray(inputs["v3"]),
    )
    Wop, bop = _fold_bn(
        np.asarray(inputs["Wo"]), np.asarray(inputs["bo"]),
        np.asarray(inputs["go"]), np.asarray(inputs["beo"]),
        np.asarray(inputs["mo"]), np.asarray(inputs["vo"]),
    )

    # ---- whole-segment sharding by sorted-width round-robin rank ----
    counts = np.bincount(batch, minlength=NSEG).astype(np.int64)
    assert np.all(batch[:-1] <= batch[1:]), "batch must be sorted"
    order = np.argsort(-counts, kind="stable")  # segment ids, width desc
    slot_w = np.maximum(counts[order[::NCORES][:S]], 1)  # width of rank 8k
    tiles, cols = _plan_tiles(slot_w)

    key = (cols, tuple(slot_w.tolist()), LDW_PAD, R1B)
    if key not in _compiled_cache:
        _compiled_cache[key] = _build_program(tiles, cols)
    nc = _compiled_cache[key]

    # column start and padded width of each slot
    slot_col = np.zeros(S, dtype=np.int64)
    slot_wt = np.zeros(S, dtype=np.int64)
    for k0, d, wt, col0 in tiles:
        for j in range(d):
            slot_col[k0 + j] = col0 + j * wt
            slot_wt[k0 + j] = wt

    starts = np.searchsorted(batch, np.arange(NSEG), side="left")

    bf = mybir.dt.np(mybir.dt.bfloat16)
    W1bf = W1p.astype(bf).astype(np.float32)
    W2bf = W2p.astype(bf).astype(np.float32)
    W3bf = W3p.astype(bf).astype(np.float32)

    in_maps = []
    core_segs = []
    for c in range(NCORES):
        segs = order[np.arange(S) * NCORES + c]  # this core's segment ids
        core_segs.append(segs)
        src = np.zeros(cols, dtype=np.int64)
        emptyc = np.zeros(cols, dtype=bool)
        for k in range(S):
            s = segs[k]
            cnt = int(counts[s])
            c0 = slot_col[k]
            wt = int(slot_wt[k])
            if cnt:
                src[c0 : c0 + cnt] = np.arange(starts[s], starts[s] + cnt)
                # dup-pad with the segment's first node
                src[c0 + cnt : c0 + wt] = starts[s]
            else:
                emptyc[c0 : c0 + wt] = True
        xTc = x[src].T.astype(bf)
        if emptyc.any():
            xTc[:, emptyc] = 0
        recipc = (1.0 / np.maximum(counts[segs], 1.0)).astype(np.float32)
        in_maps.append(
            dict(
                xT=np.ascontiguousarray(xTc),
                w1=np.ascontiguousarray(W1p.T).astype(bf),
                w2=np.ascontiguousarray(W2p.T).astype(bf),
                w3=np.ascontiguousarray(W3p.T).astype(bf),
                b1=np.ascontiguousarray(b1p[:, None]),
                b2=np.ascontiguousarray(b2p[:, None]),
                b3=np.ascontiguousarray(b3p[:, None]),
                wsum=np.ascontiguousarray(Wop[:, 0:H].T),
                wmax=np.ascontiguousarray(Wop[:, H : 2 * H].T),
                wmean=np.ascontiguousarray(Wop[:, 2 * H : 3 * H].T),
                bo=np.ascontiguousarray(bop[None, :]),
                recip=np.ascontiguousarray(recipc.reshape(S // H, H).T),
            )
        )

    ncores_run = int(os.environ.get("KERNEL_NCORES", str(NCORES)))
    res = bass_utils.run_bass_kernel_spmd(
        nc,
        in_maps[:ncores_run],
        core_ids=list(range(ncores_run)),
        trace=bool(int(os.environ.get("KERNEL_TRACE", "0"))),
        tmpdir=os.environ.get("KERNEL_TRACE_DIR") or None,
    )
    kernel.last_results = res

    # ---- host-side dup-pad correction ----
    # Replay the bf16 encoder for every segment's first node, matching the
    # device values (bf16 weights/activations, fp32 accumulate) to ~1ulp.
    first = x[starts[order[: S * NCORES]].clip(0)]  # [S*NCORES, C] rank order
    xf = first.astype(bf).astype(np.float32)
    h1f = np.maximum(xf @ W1bf.T + b1p, 0.0).astype(bf).astype(np.float32)
    h2f = np.maximum(h1f @ W2bf.T + b2p, 0.0).astype(bf).astype(np.float32)
    h3f = np.maximum(h2f @ W3bf.T + b3p, 0.0).astype(bf).astype(np.float32)

    out_full = np.zeros((NSEG, O), dtype=np.float32)
    ranks = np.arange(S)
    for c in range(ncores_run):
        segs = core_segs[c]
        o = np.array(res.results[c]["out"], dtype=np.float32)
        npads = (slot_wt - counts[segs]).astype(np.float32)
        h3c = h3f[ranks * NCORES + c]  # [S, H] first-node h3 per slot
        recipc = 1.0 / np.maximum(counts[segs], 1.0)
        corr = (h3c @ Wop[:, 0:H].T) * npads[:, None]
        corr += (h3c @ Wop[:, 2 * H : 3 * H].T) * (npads * recipc)[:, None]
        o -= corr
        empty = counts[segs] == 0
        if empty.any():
            o[empty] = bop[None, :]
        out_full[segs] = o
    return out_full



# revision 20
# speedup vs baseline: 1.1745x; 1.0639x over previous
"""DeepSets segment-reduce kernel for 8x Trainium2 NeuronCores.

Strategy (all shapes hardcoded for N=500000, C=H=128, O=64, NSEG=2048):
  - Transposed activation layout: features on SBUF partitions, nodes on the
    free axis, so segment reductions are free-axis DVE reduces.
  - Whole-segment sharding: every segment is assigned entirely to one core,
    round-robin by global sorted-width rank.  All 8 cores share an identical
    compile-time slot/tile geometry (SPMD-safe).  No collective is needed -
    the host gather is the unshard.
  - Encoder BN is folded into the linear weights; the whole encoder path is
    bf16 (inputs, weights, activations) so the PE gets fast weight loads,
    input DMA halves, and SBUF pressure drops.  PSUM stays fp32.
  - Pad columns DUPLICATE the slot's first real column.  The segment max is
    then exact on device; the known dup contribution to the segment sum is
    subtracted on the host (the host replays the bf16 encoder for each
    segment's first node, bit-matching the device values to ~1ulp).
  - Wide tiles: slots are packed D-at-a-time into up-to-1024-column tiles of
    uniform padded width, so each relu / pairwise-TT / reduce is a single
    instruction over a two-PSUM-bank access pattern - half the instruction
    and semaphore count of 512-wide tiles.  Matmuls split at the 512-column
    PSUM bank boundary.
  - Engine balance: ACT runs relu1+relu3 (+4/9 of relu2), DVE runs the rest
    of relu2, the 2x_1p bf16 pairwise pre-halving tensor_tensors, and the
    1x-locked final reduces on the halved inputs.
  - Software pipelining: tile t's p3 consumers are deferred into tile t+1
    and split by engine: relu3(t) is issued right after mm1(t+1) so it plugs
    the ACT queue while mm1 runs, and the reduces of t are issued after
    mm3(t+1) so they never head-of-line-block a ready relu1 on either
    strict-FIFO queue.
  - Final projection out = [sum|max|mean] @ Wo'.T + bo' runs per core on its
    own 256 segments; mean is handled by projecting sums through the mean
    block of Wo' and row-scaling by 1/count.
"""

import os
import sys

import numpy as np

if "/opt/trn_rl_repo" not in sys.path:
    sys.path.insert(0, "/opt/trn_rl_repo")

import concourse.bacc as bacc
import concourse.mybir as mybir
import concourse.tile as tile
from concourse import bass_utils

EPS = 1e-5
NSEG = 2048
NCORES = 8
C = 128
H = 128
O = 64
S = NSEG // NCORES  # segment slots per core (256)
WIDE = 1024  # two PSUM banks of fp32
# Idempotent LDWEIGHTS padding per matmul pair: keeps the PE array active so
# the HAM clock gate holds K=8/8 (2.4 GHz) instead of oscillating to 1.2 GHz
# during the per-tile PE idle gaps.  Each dummy load streams w1s through the
# array (~107 ns busy, no PSUM write) and is overwritten by the next real
# matmul's own self-loading weights.
LDW_PAD = int(os.environ.get("KERNEL_LDW_PAD", "0"))
# relu1 ACT/DVE boundary; 512 = baseline single two-bank p1 tile.  Values in
# [402, 512) split p1 into two single-bank PSUM tiles so each engine reads
# within one bank (bank-crossing PSUM APs are slow).
R1B = int(os.environ.get("KERNEL_R1B", "512"))

_compiled_cache = {}


def _fold_bn(W, b, g, be, m, v):
    a = g / np.sqrt(v + EPS)
    Wp = W * a[:, None]
    bp = (b - m) * a + be
    return Wp.astype(np.float32), bp.astype(np.float32)


def _plan_tiles(slot_w):
    """Pack slots (widths descending) into <=WIDE-column tiles of uniform
    padded width (multiple of 4 so bf16 half-offsets stay 4B-aligned for the
    DVE 2x_1p mode).  Returns (slot_start, n_slots, width, col_start) tiles
    plus total columns."""
    tiles = []
    col = 0
    k = 0
    n = len(slot_w)
    while k < n:
        wt = (int(slot_w[k]) + 3) & ~3
        assert 0 < wt <= WIDE // 2, f"slot width {wt} unsupported"
        d = min(WIDE // wt, n - k)
        tiles.append((k, d, wt, col))
        col += d * wt
        k += d
    return tiles, col


def _build_program(tiles, cols):
    """Emit the Bass/Tile program shared by all 8 cores."""
    nc = bacc.Bacc(
        "TRN2",
        target_bir_lowering=False,
        debug=False,
        num_devices=NCORES,
    )
    f32 = mybir.dt.float32
    bf16 = mybir.dt.bfloat16

    xT = nc.dram_tensor("xT", [C, cols], bf16, kind="ExternalInput").ap()
    w1 = nc.dram_tensor("w1", [C, H], bf16, kind="ExternalInput").ap()
    w2 = nc.dram_tensor("w2", [H, H], bf16, kind="ExternalInput").ap()
    w3 = nc.dram_tensor("w3", [H, H], bf16, kind="ExternalInput").ap()
    b1 = nc.dram_tensor("b1", [H, 1], f32, kind="ExternalInput").ap()
    b2 = nc.dram_tensor("b2", [H, 1], f32, kind="ExternalInput").ap()
    b3 = nc.dram_tensor("b3", [H, 1], f32, kind="ExternalInput").ap()
    wsum = nc.dram_tensor("wsum", [H, O], f32, kind="ExternalInput").ap()
    wmax = nc.dram_tensor("wmax", [H, O], f32, kind="ExternalInput").ap()
    wmean = nc.dram_tensor("wmean", [H, O], f32, kind="ExternalInput").ap()
    bo = nc.dram_tensor("bo", [1, O], f32, kind="ExternalInput").ap()
    # column ch holds the reciprocals for segment chunk ch (128 slots each)
    recip = nc.dram_tensor("recip", [H, S // H], f32, kind="ExternalInput").ap()
    out = nc.dram_tensor("out", [S, O], f32, kind="ExternalOutput").ap()

    relu = mybir.ActivationFunctionType.Relu
    add = mybir.AluOpType.add
    amax = mybir.AluOpType.max

    with tile.TileContext(nc) as tc:
        with (
            tc.tile_pool(name="const", bufs=1) as cpool,
            tc.tile_pool(name="xin", bufs=6) as xpool,
            tc.tile_pool(name="h1", bufs=2) as h1pool,
            tc.tile_pool(name="h2", bufs=2) as h2pool,
            tc.tile_pool(name="h3", bufs=3) as h3pool,
            tc.tile_pool(name="hm", bufs=3) as hmpool,
            tc.tile_pool(name="ht", bufs=3) as htpool,
            tc.tile_pool(name="acc", bufs=1) as accpool,
            tc.tile_pool(name="ps1", bufs=1, space="PSUM") as ps1,
            tc.tile_pool(name="ps2", bufs=1, space="PSUM") as ps2,
            tc.tile_pool(name="ps3", bufs=2, space="PSUM") as ps3,
        ):
            w1s = cpool.tile([C, H], bf16, tag="w1")
            w2s = cpool.tile([H, H], bf16, tag="w2")
            w3s = cpool.tile([H, H], bf16, tag="w3")
            b1s = cpool.tile([H, 1], f32, tag="b1")
            b2s = cpool.tile([H, 1], f32, tag="b2")
            b3s = cpool.tile([H, 1], f32, tag="b3")
            wsums = cpool.tile([H, O], f32, tag="wsum")
            wmaxs = cpool.tile([H, O], f32, tag="wmax")
            wmeans = cpool.tile([H, O], f32, tag="wmean")
            bos = cpool.tile([1, O], f32, tag="bo")
            recs = cpool.tile([H, S // H], f32, tag="recip")
            ones = cpool.tile([1, H], f32, tag="ones")

            nc.sync.dma_start(w1s[:], w1)
            nc.sync.dma_start(w2s[:], w2)
            nc.sync.dma_start(w3s[:], w3)
            nc.sync.dma_start(b1s[:], b1)
            nc.sync.dma_start(b2s[:], b2)
            nc.sync.dma_start(b3s[:], b3)
            nc.sync.dma_start(wsums[:], wsum)
            nc.sync.dma_start(wmaxs[:], wmax)
            nc.sync.dma_start(wmeans[:], wmean)
            nc.sync.dma_start(bos[:], bo)
            nc.sync.dma_start(recs[:], recip)
            nc.vector.memset(ones[:], 1.0)

            # Persistent per-slot partials (both post-relu, bias included).
            sumP = accpool.tile([H, S], f32, tag="sumP")
            maxP = accpool.tile([H, S], f32, tag="maxP")

            def consume_p3_relu(p3w, h3w, k0, d, wt, tcols):
                nc.scalar.activation(
                    h3w[:, :tcols], p3w[:, :tcols], relu, bias=b3s[:]
                )

            def consume_p3_reduce(p3w, h3w, k0, d, wt, tcols):
                h3v = h3w[:, :tcols].rearrange("p (d w) -> p d w", d=d)
                hw = wt // 2
                # DVE pre-halves both reduce inputs with 2x_1p bf16
                # tensor_tensor, then the 1x-locked reduce sees half the
                # columns.
                hm = hmpool.tile([H, WIDE // 2], bf16, tag="hm")
                hmv = hm[:, : d * hw].rearrange("p (d w) -> p d w", d=d)
                nc.vector.tensor_tensor(
                    hmv, h3v[:, :, :hw], h3v[:, :, hw:wt], op=amax
                )
                nc.vector.reduce_max(
                    maxP[:, k0 : k0 + d], hmv, axis=mybir.AxisListType.X
                )
                ht = htpool.tile([H, WIDE // 2], bf16, tag="ht")
                htv = ht[:, : d * hw].rearrange("p (d w) -> p d w", d=d)
                nc.vector.tensor_tensor(
                    htv, h3v[:, :, :hw], h3v[:, :, hw:wt], op=add
                )
                nc.vector.reduce_sum(
                    sumP[:, k0 : k0 + d], htv, axis=mybir.AxisListType.X
                )

            prev = None
            for ti, (k0, d, wt, col0) in enumerate(tiles):
                tcols = d * wt
                xt = xpool.tile([C, WIDE], bf16, tag="xt")
                nc.sync.dma_start(xt[:, :tcols], xT[:, col0 : col0 + tcols])

                s0 = min(tcols, 512)
                split = R1B != 512
                b1x = max(min(R1B, s0), tcols - 512) if tcols > 512 else tcols
                if split:
                    # two single-bank p1 tiles: the ACT/DVE boundary can sit
                    # anywhere in [tcols-512, 512] without bank-crossing APs
                    p1aw = ps1.tile([H, 512], f32, tag="p1a")
                    p1a = p1aw[:, :b1x]
                    xa = xt[:, :b1x]
                    if tcols > b1x:
                        p1bw = ps1.tile([H, 512], f32, tag="p1b")
                        p1b = p1bw[:, : tcols - b1x]
                        xb = xt[:, b1x:tcols]
                    else:
                        p1b = None
                        xb = None
                else:
                    p1 = ps1.tile([H, WIDE], f32, tag="p1")
                    p1a = p1[:, :s0]
                    p1b = p1[:, 512:tcols] if tcols > 512 else None
                    xa = xt[:, :s0]
                    xb = xt[:, 512:tcols] if tcols > 512 else None
                nc.tensor.matmul(p1a, w1s[:], xa)
                if p1b is not None:
                    nc.tensor.matmul(p1b, w1s[:], xb)
                for _ in range(LDW_PAD):
                    nc.tensor.ldweights(w1s[:, :4])
                if prev is not None:
                    consume_p3_relu(*prev)
                h1 = h1pool.tile([H, WIDE], bf16, tag="h1")
                # relu1 halves run on ACT and DVE in parallel so p1 (the
                # single-buffered stage gating mm1 of the next tile) frees
                # in one half-pass latency.
                nc.scalar.activation(
                    h1[:, :b1x] if split else h1[:, :s0],
                    p1a, relu, bias=b1s[:],
                )
                if p1b is not None:
                    nc.vector.tensor_scalar(
                        h1[:, b1x:tcols] if split else h1[:, 512:tcols],
                        p1b, b1s[:], 0.0,
                        op0=add, op1=amax,
                    )

                p2 = ps2.tile([H, WIDE], f32, tag="p2")
                nc.tensor.matmul(p2[:, :s0], w2s[:], h1[:, :s0])
                if tcols > 512:
                    nc.tensor.matmul(p2[:, 512:tcols], w2s[:], h1[:, 512:tcols])
                for _ in range(LDW_PAD):
                    nc.tensor.ldweights(w1s[:, :4])
                h2 = h2pool.tile([H, WIDE], bf16, tag="h2")
                nc.scalar.activation(h2[:, :tcols], p2[:, :tcols], relu, bias=b2s[:])

                p3 = ps3.tile([H, WIDE], f32, tag="p3")
                nc.tensor.matmul(p3[:, :s0], w3s[:], h2[:, :s0])
                if tcols > 512:
                    nc.tensor.matmul(p3[:, 512:tcols], w3s[:], h2[:, 512:tcols])

                # Deferred consumption of the PREVIOUS tile's p3 keeps the
                # ACT queue from head-of-line-blocking on the 3-matmul chain.
                if prev is not None:
                    consume_p3_reduce(*prev)
                h3 = h3pool.tile([H, WIDE], bf16, tag="h3")
                prev = (p3, h3, k0, d, wt, tcols)
            consume_p3_relu(*prev)
            consume_p3_reduce(*prev)

            # ---- epilogue: out[k, :] = sum_k @ Wsum + max_k @ Wmax
            #                + (sum_k * recip_k) @ Wmean + bo ----
            for ch in range(S // H):  # 2 chunks of 128 segments
                sl = slice(ch * H, (ch + 1) * H)
                if R1B != 512:
                    pow_ = ps1.tile([H, 512], f32, tag="p1a")
                else:
                    pow_ = ps1.tile([H, WIDE], f32, tag="p1")
                po = pow_[:, :O]
                nc.tensor.matmul(po[:], sumP[:, sl], wsums[:], start=True, stop=False)
                nc.tensor.matmul(po[:], maxP[:, sl], wmaxs[:], start=False, stop=False)
                nc.tensor.matmul(po[:], ones[:], bos[:], start=False, stop=True)

                pmw = ps2.tile([H, WIDE], f32, tag="p2")
                pm = pmw[:, :O]
                nc.tensor.matmul(pm[:], sumP[:, sl], wmeans[:], start=True, stop=True)

                om = h1pool.tile([H, O], f32, tag="om")
                nc.vector.tensor_scalar_mul(om[:], pm[:], recs[:, ch : ch + 1])
                ot = h2pool.tile([H, O], f32, tag="ot")
                nc.vector.tensor_tensor(ot[:], po[:], om[:], op=add)
                nc.sync.dma_start(out[sl, :], ot[:])

    nc.compile()
    return nc


def kernel(**inputs):
    x = np.asarray(inputs["x"], dtype=np.float32)
    batch = np.asarray(inputs["batch"]).astype(np.int64)

    # ---- fold BN into the linears ----
    W1p, b1p = _fold_bn(
        np.asarray(inputs["W1"]), np.asarray(inputs["b1"]),
        np.asarray(inputs["g1"]), np.asarray(inputs["be1"]),
        np.asarray(inputs["m1"]), np.asarray(inputs["v1"]),
    )
    W2p, b2p = _fold_bn(
        np.asarray(inputs["W2"]), np.asarray(inputs["b2"]),
        np.asarray(inputs["g2"]), np.asarray(inputs["be2"]),
        np.asarray(inputs["m2"]), np.asarray(inputs["v2"]),
    )
    W3p, b3p = _fold_bn(
        np.asarray(inputs["W3"]), np.asarray(inputs["b3"]),
        np.asarray(inputs["g3"]), np.asarray(inputs["be3"]),
        np.asarray(inputs["m3"]), np.asarray(inputs["v3"]),
    )
    Wop, bop = _fold_bn(
        np.asarray(inputs["Wo"]), np.asarray(inputs["bo"]),
        np.asarray(inputs["go"]), np.asarray(inputs["beo"]),
        np.asarray(inputs["mo"]), np.asarray(inputs["vo"]),
    )

    # ---- whole-segment sharding by sorted-width round-robin rank ----
    counts = np.bincount(batch, minlength=NSEG).astype(np.int64)
    assert np.all(batch[:-1] <= batch[1:]), "batch must be sorted"
    order = np.argsort(-counts, kind="stable")  # segment ids, width desc
    slot_w = np.maximum(counts[order[::NCORES][:S]], 1)  # width of rank 8k
    tiles, cols = _plan_tiles(slot_w)

    key = (cols, tuple(slot_w.tolist()), LDW_PAD, R1B)
    if key not in _compiled_cache:
        _compiled_cache[key] = _build_program(tiles, cols)
    nc = _compiled_cache[key]

    # column start and padded width of each slot
    slot_col = np.zeros(S, dtype=np.int64)
    slot_wt = np.zeros(S, dtype=np.int64)
    for k0, d, wt, col0 in tiles:
        for j in range(d):
            slot_col[k0 + j] = col0 + j * wt
            slot_wt[k0 + j] = wt

    starts = np.searchsorted(batch, np.arange(NSEG), side="left")

    bf = mybir.dt.np(mybir.dt.bfloat16)
    W1bf = W1p.astype(bf).astype(np.float32)
    W2bf = W2p.astype(bf).astype(np.float32)
    W3bf = W3p.astype(bf).astype(np.float32)

    in_maps = []
    core_segs = []
    for c in range(NCORES):
        segs = order[np.arange(S) * NCORES + c]  # this core's segment ids
        core_segs.append(segs)
        src = np.zeros(cols, dtype=np.int64)
        emptyc = np.zeros(cols, dtype=bool)
        for k in range(S):
            s = segs[k]
            cnt = int(counts[s])
            c0 = slot_col[k]
            wt = int(slot_wt[k])
            if cnt:
                src[c0 : c0 + cnt] = np.arange(starts[s], starts[s] + cnt)
                # dup-pad with the segment's first node
                src[c0 + cnt : c0 + wt] = starts[s]
            else:
                emptyc[c0 : c0 + wt] = True
        xTc = x[src].T.astype(bf)
        if emptyc.any():
            xTc[:, emptyc] = 0
        recipc = (1.0 / np.maximum(counts[segs], 1.0)).astype(np.float32)
        in_maps.append(
            dict(
                xT=np.ascontiguousarray(xTc),
                w1=np.ascontiguousarray(W1p.T).astype(bf),
                w2=np.ascontiguousarray(W2p.T).astype(bf),
                w3=np.ascontiguousarray(W3p.T).astype(bf),
                b1=np.ascontiguousarray(b1p[:, None]),
                b2=np.ascontiguousarray(b2p[:, None]),
                b3=np.ascontiguousarray(b3p[:, None]),
                wsum=np.ascontiguousarray(Wop[:, 0:H].T),
                wmax=np.ascontiguousarray(Wop[:, H : 2 * H].T),
                wmean=np.ascontiguousarray(Wop[:, 2 * H : 3 * H].T),
                bo=np.ascontiguousarray(bop[None, :]),
                recip=np.ascontiguousarray(recipc.reshape(S // H, H).T),
            )
        )

    ncores_run = int(os.environ.get("KERNEL_NCORES", str(NCORES)))
    res = bass_utils.run_bass_kernel_spmd(
        nc,
        in_maps[:ncores_run],
        core_ids=list(range(ncores_run)),
        trace=bool(int(os.environ.get("KERNEL_TRACE", "0"))),
        tmpdir=os.environ.get("KERNEL_TRACE_DIR") or None,
    )
    kernel.last_results = res

    # ---- host-side dup-pad correction ----
    # Replay the bf16 encoder for every segment's first node, matching the
    # device values (bf16 weights/activations, fp32 accumulate) to ~1ulp.
    first = x[starts[order[: S * NCORES]].clip(0)]  # [S*NCORES, C] rank order
    xf = first.astype(bf).astype(np.float32)
    h1f = np.maximum(xf @ W1bf.T + b1p, 0.0).astype(bf).astype(np.float32)
    h2f = np.maximum(h1f @ W2bf.T + b2p, 0.0).astype(bf).astype(np.float32)
    h3f = np.maximum(h2f @ W3bf.T + b3p, 0.0).astype(bf).astype(np.float32)

    out_full = np.zeros((NSEG, O), dtype=np.float32)
    ranks = np.arange(S)
    for c in range(ncores_run):
        segs = core_segs[c]
        o = np.array(res.results[c]["out"], dtype=np.float32)
        npads = (slot_wt - counts[segs]).astype(np.float32)
        h3c = h3f[ranks * NCORES + c]  # [S, H] first-node h3 per slot
        recipc = 1.0 / np.maximum(counts[segs], 1.0)
        corr = (h3c @ Wop[:, 0:H].T) * npads[:, None]
        corr += (h3c @ Wop[:, 2 * H : 3 * H].T) * (npads * recipc)[:, None]
        o -= corr
        empty = counts[segs] == 0
        if empty.any():
            o[empty] = bop[None, :]
        out_full[segs] = o
    return out_full



# revision 21
# speedup vs baseline: 1.1759x; 1.0013x over previous
"""DeepSets segment-reduce kernel for 8x Trainium2 NeuronCores.

Strategy (all shapes hardcoded for N=500000, C=H=128, O=64, NSEG=2048):
  - Transposed activation layout: features on SBUF partitions, nodes on the
    free axis, so segment reductions are free-axis DVE reduces.
  - Whole-segment sharding: every segment is assigned entirely to one core,
    round-robin by global sorted-width rank.  All 8 cores share an identical
    compile-time slot/tile geometry (SPMD-safe).  No collective is needed -
    the host gather is the unshard.
  - Encoder BN is folded into the linear weights; the whole encoder path is
    bf16 (inputs, weights, activations) so the PE gets fast weight loads,
    input DMA halves, and SBUF pressure drops.  PSUM stays fp32.
  - Pad columns DUPLICATE the slot's first real column.  The segment max is
    then exact on device; the known dup contribution to the segment sum is
    subtracted on the host (the host replays the bf16 encoder for each
    segment's first node, bit-matching the device values to ~1ulp).
  - Wide tiles: slots are packed D-at-a-time into up-to-1024-column tiles of
    uniform padded width, so each relu / pairwise-TT / reduce is a single
    instruction over a two-PSUM-bank access pattern - half the instruction
    and semaphore count of 512-wide tiles.  Matmuls split at the 512-column
    PSUM bank boundary.
  - Engine balance: ACT runs relu1+relu3 (+4/9 of relu2), DVE runs the rest
    of relu2, the 2x_1p bf16 pairwise pre-halving tensor_tensors, and the
    1x-locked final reduces on the halved inputs.
  - Software pipelining: tile t's p3 consumers are deferred into tile t+1
    and split by engine: relu3(t) is issued right after mm1(t+1) so it plugs
    the ACT queue while mm1 runs, and the reduces of t are issued after
    mm3(t+1) so they never head-of-line-block a ready relu1 on either
    strict-FIFO queue.
  - Final projection out = [sum|max|mean] @ Wo'.T + bo' runs per core on its
    own 256 segments; mean is handled by projecting sums through the mean
    block of Wo' and row-scaling by 1/count.
"""

import os
import sys

import numpy as np

if "/opt/trn_rl_repo" not in sys.path:
    sys.path.insert(0, "/opt/trn_rl_repo")

import concourse.bacc as bacc
import concourse.mybir as mybir
import concourse.tile as tile
from concourse import bass_utils

EPS = 1e-5
NSEG = 2048
NCORES = 8
C = 128
H = 128
O = 64
S = NSEG // NCORES  # segment slots per core (256)
WIDE = 1024  # two PSUM banks of fp32
# Idempotent LDWEIGHTS padding per matmul pair: keeps the PE array active so
# the HAM clock gate holds K=8/8 (2.4 GHz) instead of oscillating to 1.2 GHz
# during the per-tile PE idle gaps.  Each dummy load streams w1s through the
# array (~107 ns busy, no PSUM write) and is overwritten by the next real
# matmul's own self-loading weights.
LDW_PAD = int(os.environ.get("KERNEL_LDW_PAD", "0"))
# relu1 ACT/DVE boundary; 512 = baseline single two-bank p1 tile.  Values in
# [402, 512) split p1 into two single-bank PSUM tiles so each engine reads
# within one bank (bank-crossing PSUM APs are slow).
R1B = int(os.environ.get("KERNEL_R1B", "480"))

_compiled_cache = {}


def _fold_bn(W, b, g, be, m, v):
    a = g / np.sqrt(v + EPS)
    Wp = W * a[:, None]
    bp = (b - m) * a + be
    return Wp.astype(np.float32), bp.astype(np.float32)


def _plan_tiles(slot_w):
    """Pack slots (widths descending) into <=WIDE-column tiles of uniform
    padded width (multiple of 4 so bf16 half-offsets stay 4B-aligned for the
    DVE 2x_1p mode).  Returns (slot_start, n_slots, width, col_start) tiles
    plus total columns."""
    tiles = []
    col = 0
    k = 0
    n = len(slot_w)
    while k < n:
        wt = (int(slot_w[k]) + 3) & ~3
        assert 0 < wt <= WIDE // 2, f"slot width {wt} unsupported"
        d = min(WIDE // wt, n - k)
        tiles.append((k, d, wt, col))
        col += d * wt
        k += d
    return tiles, col


def _build_program(tiles, cols):
    """Emit the Bass/Tile program shared by all 8 cores."""
    nc = bacc.Bacc(
        "TRN2",
        target_bir_lowering=False,
        debug=False,
        num_devices=NCORES,
    )
    f32 = mybir.dt.float32
    bf16 = mybir.dt.bfloat16

    xT = nc.dram_tensor("xT", [C, cols], bf16, kind="ExternalInput").ap()
    w1 = nc.dram_tensor("w1", [C, H], bf16, kind="ExternalInput").ap()
    w2 = nc.dram_tensor("w2", [H, H], bf16, kind="ExternalInput").ap()
    w3 = nc.dram_tensor("w3", [H, H], bf16, kind="ExternalInput").ap()
    b1 = nc.dram_tensor("b1", [H, 1], f32, kind="ExternalInput").ap()
    b2 = nc.dram_tensor("b2", [H, 1], f32, kind="ExternalInput").ap()
    b3 = nc.dram_tensor("b3", [H, 1], f32, kind="ExternalInput").ap()
    wsum = nc.dram_tensor("wsum", [H, O], f32, kind="ExternalInput").ap()
    wmax = nc.dram_tensor("wmax", [H, O], f32, kind="ExternalInput").ap()
    wmean = nc.dram_tensor("wmean", [H, O], f32, kind="ExternalInput").ap()
    bo = nc.dram_tensor("bo", [1, O], f32, kind="ExternalInput").ap()
    # column ch holds the reciprocals for segment chunk ch (128 slots each)
    recip = nc.dram_tensor("recip", [H, S // H], f32, kind="ExternalInput").ap()
    out = nc.dram_tensor("out", [S, O], f32, kind="ExternalOutput").ap()

    relu = mybir.ActivationFunctionType.Relu
    add = mybir.AluOpType.add
    amax = mybir.AluOpType.max

    with tile.TileContext(nc) as tc:
        with (
            tc.tile_pool(name="const", bufs=1) as cpool,
            tc.tile_pool(name="xin", bufs=6) as xpool,
            tc.tile_pool(name="h1", bufs=2) as h1pool,
            tc.tile_pool(name="h2", bufs=2) as h2pool,
            tc.tile_pool(name="h3", bufs=3) as h3pool,
            tc.tile_pool(name="hm", bufs=3) as hmpool,
            tc.tile_pool(name="ht", bufs=3) as htpool,
            tc.tile_pool(name="acc", bufs=1) as accpool,
            tc.tile_pool(name="ps1", bufs=1, space="PSUM") as ps1,
            tc.tile_pool(name="ps2", bufs=1, space="PSUM") as ps2,
            tc.tile_pool(name="ps3", bufs=2, space="PSUM") as ps3,
        ):
            w1s = cpool.tile([C, H], bf16, tag="w1")
            w2s = cpool.tile([H, H], bf16, tag="w2")
            w3s = cpool.tile([H, H], bf16, tag="w3")
            b1s = cpool.tile([H, 1], f32, tag="b1")
            b2s = cpool.tile([H, 1], f32, tag="b2")
            b3s = cpool.tile([H, 1], f32, tag="b3")
            wsums = cpool.tile([H, O], f32, tag="wsum")
            wmaxs = cpool.tile([H, O], f32, tag="wmax")
            wmeans = cpool.tile([H, O], f32, tag="wmean")
            bos = cpool.tile([1, O], f32, tag="bo")
            recs = cpool.tile([H, S // H], f32, tag="recip")
            ones = cpool.tile([1, H], f32, tag="ones")

            nc.sync.dma_start(w1s[:], w1)
            nc.sync.dma_start(w2s[:], w2)
            nc.sync.dma_start(w3s[:], w3)
            nc.sync.dma_start(b1s[:], b1)
            nc.sync.dma_start(b2s[:], b2)
            nc.sync.dma_start(b3s[:], b3)
            nc.sync.dma_start(wsums[:], wsum)
            nc.sync.dma_start(wmaxs[:], wmax)
            nc.sync.dma_start(wmeans[:], wmean)
            nc.sync.dma_start(bos[:], bo)
            nc.sync.dma_start(recs[:], recip)
            nc.vector.memset(ones[:], 1.0)

            # Persistent per-slot partials (both post-relu, bias included).
            sumP = accpool.tile([H, S], f32, tag="sumP")
            maxP = accpool.tile([H, S], f32, tag="maxP")

            def consume_p3_relu(p3w, h3w, k0, d, wt, tcols):
                nc.scalar.activation(
                    h3w[:, :tcols], p3w[:, :tcols], relu, bias=b3s[:]
                )

            def consume_p3_reduce(p3w, h3w, k0, d, wt, tcols):
                h3v = h3w[:, :tcols].rearrange("p (d w) -> p d w", d=d)
                hw = wt // 2
                # DVE pre-halves both reduce inputs with 2x_1p bf16
                # tensor_tensor, then the 1x-locked reduce sees half the
                # columns.
                hm = hmpool.tile([H, WIDE // 2], bf16, tag="hm")
                hmv = hm[:, : d * hw].rearrange("p (d w) -> p d w", d=d)
                nc.vector.tensor_tensor(
                    hmv, h3v[:, :, :hw], h3v[:, :, hw:wt], op=amax
                )
                nc.vector.reduce_max(
                    maxP[:, k0 : k0 + d], hmv, axis=mybir.AxisListType.X
                )
                ht = htpool.tile([H, WIDE // 2], bf16, tag="ht")
                htv = ht[:, : d * hw].rearrange("p (d w) -> p d w", d=d)
                nc.vector.tensor_tensor(
                    htv, h3v[:, :, :hw], h3v[:, :, hw:wt], op=add
                )
                nc.vector.reduce_sum(
                    sumP[:, k0 : k0 + d], htv, axis=mybir.AxisListType.X
                )

            prev = None
            for ti, (k0, d, wt, col0) in enumerate(tiles):
                tcols = d * wt
                xt = xpool.tile([C, WIDE], bf16, tag="xt")
                nc.sync.dma_start(xt[:, :tcols], xT[:, col0 : col0 + tcols])

                s0 = min(tcols, 512)
                split = R1B != 512
                b1x = max(min(R1B, s0), tcols - 512) if tcols > 512 else tcols
                if split:
                    # two single-bank p1 tiles: the ACT/DVE boundary can sit
                    # anywhere in [tcols-512, 512] without bank-crossing APs
                    p1aw = ps1.tile([H, 512], f32, tag="p1a")
                    p1a = p1aw[:, :b1x]
                    xa = xt[:, :b1x]
                    if tcols > b1x:
                        p1bw = ps1.tile([H, 512], f32, tag="p1b")
                        p1b = p1bw[:, : tcols - b1x]
                        xb = xt[:, b1x:tcols]
                    else:
                        p1b = None
                        xb = None
                else:
                    p1 = ps1.tile([H, WIDE], f32, tag="p1")
                    p1a = p1[:, :s0]
                    p1b = p1[:, 512:tcols] if tcols > 512 else None
                    xa = xt[:, :s0]
                    xb = xt[:, 512:tcols] if tcols > 512 else None
                nc.tensor.matmul(p1a, w1s[:], xa)
                if p1b is not None:
                    nc.tensor.matmul(p1b, w1s[:], xb)
                for _ in range(LDW_PAD):
                    nc.tensor.ldweights(w1s[:, :4])
                if prev is not None:
                    consume_p3_relu(*prev)
                h1 = h1pool.tile([H, WIDE], bf16, tag="h1")
                # relu1 halves run on ACT and DVE in parallel so p1 (the
                # single-buffered stage gating mm1 of the next tile) frees
                # in one half-pass latency.
                nc.scalar.activation(
                    h1[:, :b1x] if split else h1[:, :s0],
                    p1a, relu, bias=b1s[:],
                )
                if p1b is not None:
                    nc.vector.tensor_scalar(
                        h1[:, b1x:tcols] if split else h1[:, 512:tcols],
                        p1b, b1s[:], 0.0,
                        op0=add, op1=amax,
                    )

                p2 = ps2.tile([H, WIDE], f32, tag="p2")
                nc.tensor.matmul(p2[:, :s0], w2s[:], h1[:, :s0])
                if tcols > 512:
                    nc.tensor.matmul(p2[:, 512:tcols], w2s[:], h1[:, 512:tcols])
                for _ in range(LDW_PAD):
                    nc.tensor.ldweights(w1s[:, :4])
                h2 = h2pool.tile([H, WIDE], bf16, tag="h2")
                nc.scalar.activation(h2[:, :tcols], p2[:, :tcols], relu, bias=b2s[:])

                p3 = ps3.tile([H, WIDE], f32, tag="p3")
                nc.tensor.matmul(p3[:, :s0], w3s[:], h2[:, :s0])
                if tcols > 512:
                    nc.tensor.matmul(p3[:, 512:tcols], w3s[:], h2[:, 512:tcols])

                # Deferred consumption of the PREVIOUS tile's p3 keeps the
                # ACT queue from head-of-line-blocking on the 3-matmul chain.
                if prev is not None:
                    consume_p3_reduce(*prev)
                h3 = h3pool.tile([H, WIDE], bf16, tag="h3")
                prev = (p3, h3, k0, d, wt, tcols)
            consume_p3_relu(*prev)
            consume_p3_reduce(*prev)

            # ---- epilogue: out[k, :] = sum_k @ Wsum + max_k @ Wmax
            #                + (sum_k * recip_k) @ Wmean + bo ----
            for ch in range(S // H):  # 2 chunks of 128 segments
                sl = slice(ch * H, (ch + 1) * H)
                if R1B != 512:
                    pow_ = ps1.tile([H, 512], f32, tag="p1a")
                else:
                    pow_ = ps1.tile([H, WIDE], f32, tag="p1")
                po = pow_[:, :O]
                nc.tensor.matmul(po[:], sumP[:, sl], wsums[:], start=True, stop=False)
                nc.tensor.matmul(po[:], maxP[:, sl], wmaxs[:], start=False, stop=False)
                nc.tensor.matmul(po[:], ones[:], bos[:], start=False, stop=True)

                pmw = ps2.tile([H, WIDE], f32, tag="p2")
                pm = pmw[:, :O]
                nc.tensor.matmul(pm[:], sumP[:, sl], wmeans[:], start=True, stop=True)

                om = h1pool.tile([H, O], f32, tag="om")
                nc.vector.tensor_scalar_mul(om[:], pm[:], recs[:, ch : ch + 1])
                ot = h2pool.tile([H, O], f32, tag="ot")
                nc.vector.tensor_tensor(ot[:], po[:], om[:], op=add)
                nc.sync.dma_start(out[sl, :], ot[:])

    nc.compile()
    return nc


def kernel(**inputs):
    x = np.asarray(inputs["x"], dtype=np.float32)
    batch = np.asarray(inputs["batch"]).astype(np.int64)

    # ---- fold BN into the linears ----
    W1p, b1p = _fold_bn(
        np.asarray(inputs["W1"]), np.asarray(inputs["b1"]),
        np.asarray(inputs["g1"]), np.asarray(inputs["be1"]),
        np.asarray(inputs["m1"]), np.asarray(inputs["v1"]),
    )
    W2p, b2p = _fold_bn(
        np.asarray(inputs["W2"]), np.asarray(inputs["b2"]),
        np.asarray(inputs["g2"]), np.asarray(inputs["be2"]),
        np.asarray(inputs["m2"]), np.asarray(inputs["v2"]),
    )
    W3p, b3p = _fold_bn(
        np.asarray(inputs["W3"]), np.asarray(inputs["b3"]),
        np.asarray(inputs["g3"]), np.asarray(inputs["be3"]),
        np.asarray(inputs["m3"]), np.asarray(inputs["v3"]),
    )
    Wop, bop = _fold_bn(
        np.asarray(inputs["Wo"]), np.asarray(inputs["bo"]),
        np.asarray(inputs["go"]), np.asarray(inputs["beo"]),
        np.asarray(inputs["mo"]), np.asarray(inputs["vo"]),
    )

    # ---- whole-segment sharding by sorted-width round-robin rank ----
    counts = np.bincount(batch, minlength=NSEG).astype(np.int64)
    assert np.all(batch[:-1] <= batch[1:]), "batch must be sorted"
    order = np.argsort(-counts, kind="stable")  # segment ids, width desc
    slot_w = np.maximum(counts[order[::NCORES][:S]], 1)  # width of rank 8k
    tiles, cols = _plan_tiles(slot_w)

    key = (cols, tuple(slot_w.tolist()), LDW_PAD, R1B)
    if key not in _compiled_cache:
        _compiled_cache[key] = _build_program(tiles, cols)
    nc = _compiled_cache[key]

    # column start and padded width of each slot
    slot_col = np.zeros(S, dtype=np.int64)
    slot_wt = np.zeros(S, dtype=np.int64)
    for k0, d, wt, col0 in tiles:
        for j in range(d):
            slot_col[k0 + j] = col0 + j * wt
            slot_wt[k0 + j] = wt

    starts = np.searchsorted(batch, np.arange(NSEG), side="left")

    bf = mybir.dt.np(mybir.dt.bfloat16)
    W1bf = W1p.astype(bf).astype(np.float32)
    W2bf = W2p.astype(bf).astype(np.float32)
    W3bf = W3p.astype(bf).astype(np.float32)

    in_maps = []
    core_segs = []
    for c in range(NCORES):
        segs = order[np.arange(S) * NCORES + c]  # this core's segment ids
        core_segs.append(segs)
        src = np.zeros(cols, dtype=np.int64)
        emptyc = np.zeros(cols, dtype=bool)
        for k in range(S):
            s = segs[k]
            cnt = int(counts[s])
            c0 = slot_col[k]
            wt = int(slot_wt[k])
            if cnt:
                src[c0 : c0 + cnt] = np.arange(starts[s], starts[s] + cnt)
                # dup-pad with the segment's first node
                src[c0 + cnt : c0 + wt] = starts[s]
            else:
                emptyc[c0 : c0 + wt] = True
        xTc = x[src].T.astype(bf)
        if emptyc.any():
            xTc[:, emptyc] = 0
        recipc = (1.0 / np.maximum(counts[segs], 1.0)).astype(np.float32)
        in_maps.append(
            dict(
                xT=np.ascontiguousarray(xTc),
                w1=np.ascontiguousarray(W1p.T).astype(bf),
                w2=np.ascontiguousarray(W2p.T).astype(bf),
                w3=np.ascontiguousarray(W3p.T).astype(bf),
                b1=np.ascontiguousarray(b1p[:, None]),
                b2=np.ascontiguousarray(b2p[:, None]),
                b3=np.ascontiguousarray(b3p[:, None]),
                wsum=np.ascontiguousarray(Wop[:, 0:H].T),
                wmax=np.ascontiguousarray(Wop[:, H : 2 * H].T),
                wmean=np.ascontiguousarray(Wop[:, 2 * H : 3 * H].T),
                bo=np.ascontiguousarray(bop[None, :]),
                recip=np.ascontiguousarray(recipc.reshape(S // H, H).T),
            )
        )

    ncores_run = int(os.environ.get("KERNEL_NCORES", str(NCORES)))
    res = bass_utils.run_bass_kernel_spmd(
        nc,
        in_maps[:ncores_run],
        core_ids=list(range(ncores_run)),
        trace=bool(int(os.environ.get("KERNEL_TRACE", "0"))),
        tmpdir=os.environ.get("KERNEL_TRACE_DIR") or None,
    )
    kernel.last_results = res

    # ---- host-side dup-pad correction ----
    # Replay the bf16 encoder for every segment's first node, matching the
    # device values (bf16 weights/activations, fp32 accumulate) to ~1ulp.
    first = x[starts[order[: S * NCORES]].clip(0)]  # [S*NCORES, C] rank order
    xf = first.astype(bf).astype(np.float32)
    h1f = np.maximum(xf @ W1bf.T + b1p, 0.0).astype(bf).astype(np.float32)
    h2f = np.maximum(h1f @ W2bf.T + b2p, 0.0).astype(bf).astype(np.float32)
    h3f = np.maximum(h2f @ W3bf.T + b3p, 0.0).astype(bf).astype(np.float32)

    out_full = np.zeros((NSEG, O), dtype=np.float32)
    ranks = np.arange(S)
    for c in range(ncores_run):
        segs = core_segs[c]
        o = np.array(res.results[c]["out"], dtype=np.float32)
        npads = (slot_wt - counts[segs]).astype(np.float32)
        h3c = h3f[ranks * NCORES + c]  # [S, H] first-node h3 per slot
        recipc = 1.0 / np.maximum(counts[segs], 1.0)
        corr = (h3c @ Wop[:, 0:H].T) * npads[:, None]
        corr += (h3c @ Wop[:, 2 * H : 3 * H].T) * (npads * recipc)[:, None]
        o -= corr
        empty = counts[segs] == 0
        if empty.any():
            o[empty] = bop[None, :]
        out_full[segs] = o
    return out_full



# revision 22
# speedup vs baseline: 1.2218x; 1.0390x over previous
"""DeepSets segment-reduce kernel for 8x Trainium2 NeuronCores.

Strategy (all shapes hardcoded for N=500000, C=H=128, O=64, NSEG=2048):
  - Transposed activation layout: features on SBUF partitions, nodes on the
    free axis, so segment reductions are free-axis DVE reduces.
  - Whole-segment sharding: every segment is assigned entirely to one core,
    round-robin by global sorted-width rank.  All 8 cores share an identical
    compile-time slot/tile geometry (SPMD-safe).  No collective is needed -
    the host gather is the unshard.
  - Encoder BN is folded into the linear weights; the whole encoder path is
    bf16 (inputs, weights, activations) so the PE gets fast weight loads,
    input DMA halves, and SBUF pressure drops.  PSUM stays fp32.
  - Pad columns DUPLICATE the slot's first real column.  The segment max is
    then exact on device; the known dup contribution to the segment sum is
    subtracted on the host (the host replays the bf16 encoder for each
    segment's first node, bit-matching the device values to ~1ulp).
  - Wide tiles: slots are packed D-at-a-time into up-to-1024-column tiles of
    uniform padded width, so each relu / pairwise-TT / reduce is a single
    instruction over a two-PSUM-bank access pattern - half the instruction
    and semaphore count of 512-wide tiles.  Matmuls split at the 512-column
    PSUM bank boundary.
  - Engine balance: ACT runs relu1+relu3 (+4/9 of relu2), DVE runs the rest
    of relu2, the 2x_1p bf16 pairwise pre-halving tensor_tensors, and the
    1x-locked final reduces on the halved inputs.
  - Software pipelining: tile t's p3 consumers are deferred into tile t+1
    and split by engine: relu3(t) is issued right after mm1(t+1) so it plugs
    the ACT queue while mm1 runs, and the reduces of t are issued after
    mm3(t+1) so they never head-of-line-block a ready relu1 on either
    strict-FIFO queue.
  - Final projection out = [sum|max|mean] @ Wo'.T + bo' runs per core on its
    own 256 segments; mean is handled by projecting sums through the mean
    block of Wo' and row-scaling by 1/count.
"""

import os
import sys

import numpy as np

if "/opt/trn_rl_repo" not in sys.path:
    sys.path.insert(0, "/opt/trn_rl_repo")

import concourse.bacc as bacc
import concourse.mybir as mybir
import concourse.tile as tile
from concourse import bass_utils

EPS = 1e-5
NSEG = 2048
NCORES = 8
C = 128
H = 128
O = 64
S = NSEG // NCORES  # segment slots per core (256)
WIDE = 1024  # two PSUM banks of fp32
# Idempotent LDWEIGHTS padding per matmul pair: keeps the PE array active so
# the HAM clock gate holds K=8/8 (2.4 GHz) instead of oscillating to 1.2 GHz
# during the per-tile PE idle gaps.  Each dummy load streams w1s through the
# array (~107 ns busy, no PSUM write) and is overwritten by the next real
# matmul's own self-loading weights.
LDW_PAD = int(os.environ.get("KERNEL_LDW_PAD", "0"))
# relu1 ACT/DVE boundary; 512 = baseline single two-bank p1 tile.  Values in
# [402, 512) split p1 into two single-bank PSUM tiles so each engine reads
# within one bank (bank-crossing PSUM APs are slow).
R1B = int(os.environ.get("KERNEL_R1B", "511"))
# split p2 into two single-bank tiles (finer WAR granularity for mm2)
P2SPLIT = int(os.environ.get("KERNEL_P2SPLIT", "0"))

_compiled_cache = {}


def _fold_bn(W, b, g, be, m, v):
    a = g / np.sqrt(v + EPS)
    Wp = W * a[:, None]
    bp = (b - m) * a + be
    return Wp.astype(np.float32), bp.astype(np.float32)


def _plan_tiles(slot_w):
    """Pack slots (widths descending) into <=WIDE-column tiles of uniform
    padded width (multiple of 4 so bf16 half-offsets stay 4B-aligned for the
    DVE 2x_1p mode).  Returns (slot_start, n_slots, width, col_start) tiles
    plus total columns."""
    tiles = []
    col = 0
    k = 0
    n = len(slot_w)
    while k < n:
        wt = (int(slot_w[k]) + 3) & ~3
        assert 0 < wt <= WIDE // 2, f"slot width {wt} unsupported"
        d = min(WIDE // wt, n - k)
        tiles.append((k, d, wt, col))
        col += d * wt
        k += d
    return tiles, col


def _build_program(tiles, cols):
    """Emit the Bass/Tile program shared by all 8 cores."""
    nc = bacc.Bacc(
        "TRN2",
        target_bir_lowering=False,
        debug=False,
        num_devices=NCORES,
    )
    f32 = mybir.dt.float32
    bf16 = mybir.dt.bfloat16

    xT = nc.dram_tensor("xT", [C, cols], bf16, kind="ExternalInput").ap()
    w1 = nc.dram_tensor("w1", [C, H], bf16, kind="ExternalInput").ap()
    w2 = nc.dram_tensor("w2", [H, H], bf16, kind="ExternalInput").ap()
    w3 = nc.dram_tensor("w3", [H, H], bf16, kind="ExternalInput").ap()
    b1 = nc.dram_tensor("b1", [H, 1], f32, kind="ExternalInput").ap()
    b2 = nc.dram_tensor("b2", [H, 1], f32, kind="ExternalInput").ap()
    b3 = nc.dram_tensor("b3", [H, 1], f32, kind="ExternalInput").ap()
    wsum = nc.dram_tensor("wsum", [H, O], f32, kind="ExternalInput").ap()
    wmax = nc.dram_tensor("wmax", [H, O], f32, kind="ExternalInput").ap()
    wmean = nc.dram_tensor("wmean", [H, O], f32, kind="ExternalInput").ap()
    bo = nc.dram_tensor("bo", [1, O], f32, kind="ExternalInput").ap()
    # column ch holds the reciprocals for segment chunk ch (128 slots each)
    recip = nc.dram_tensor("recip", [H, S // H], f32, kind="ExternalInput").ap()
    out = nc.dram_tensor("out", [S, O], f32, kind="ExternalOutput").ap()

    relu = mybir.ActivationFunctionType.Relu
    add = mybir.AluOpType.add
    amax = mybir.AluOpType.max

    with tile.TileContext(nc) as tc:
        with (
            tc.tile_pool(name="const", bufs=1) as cpool,
            tc.tile_pool(name="xin", bufs=6) as xpool,
            tc.tile_pool(name="h1", bufs=2) as h1pool,
            tc.tile_pool(name="h2", bufs=2) as h2pool,
            tc.tile_pool(name="h3", bufs=3) as h3pool,
            tc.tile_pool(name="hm", bufs=3) as hmpool,
            tc.tile_pool(name="ht", bufs=3) as htpool,
            tc.tile_pool(name="acc", bufs=1) as accpool,
            tc.tile_pool(name="ps1", bufs=1, space="PSUM") as ps1,
            tc.tile_pool(name="ps2", bufs=1, space="PSUM") as ps2,
            tc.tile_pool(name="ps3", bufs=2, space="PSUM") as ps3,
        ):
            w1s = cpool.tile([C, H], bf16, tag="w1")
            w2s = cpool.tile([H, H], bf16, tag="w2")
            w3s = cpool.tile([H, H], bf16, tag="w3")
            b1s = cpool.tile([H, 1], f32, tag="b1")
            b2s = cpool.tile([H, 1], f32, tag="b2")
            b3s = cpool.tile([H, 1], f32, tag="b3")
            wsums = cpool.tile([H, O], f32, tag="wsum")
            wmaxs = cpool.tile([H, O], f32, tag="wmax")
            wmeans = cpool.tile([H, O], f32, tag="wmean")
            bos = cpool.tile([1, O], f32, tag="bo")
            recs = cpool.tile([H, S // H], f32, tag="recip")
            ones = cpool.tile([1, H], f32, tag="ones")

            nc.sync.dma_start(w1s[:], w1)
            nc.sync.dma_start(w2s[:], w2)
            nc.sync.dma_start(w3s[:], w3)
            nc.sync.dma_start(b1s[:], b1)
            nc.sync.dma_start(b2s[:], b2)
            nc.sync.dma_start(b3s[:], b3)
            nc.sync.dma_start(wsums[:], wsum)
            nc.sync.dma_start(wmaxs[:], wmax)
            nc.sync.dma_start(wmeans[:], wmean)
            nc.sync.dma_start(bos[:], bo)
            nc.sync.dma_start(recs[:], recip)
            nc.vector.memset(ones[:], 1.0)

            # Persistent per-slot partials (both post-relu, bias included).
            sumP = accpool.tile([H, S], f32, tag="sumP")
            maxP = accpool.tile([H, S], f32, tag="maxP")

            def consume_p3_relu(p3w, h3w, k0, d, wt, tcols):
                nc.scalar.activation(
                    h3w[:, :tcols], p3w[:, :tcols], relu, bias=b3s[:]
                )

            def consume_p3_reduce(p3w, h3w, k0, d, wt, tcols):
                h3v = h3w[:, :tcols].rearrange("p (d w) -> p d w", d=d)
                hw = wt // 2
                # DVE pre-halves both reduce inputs with 2x_1p bf16
                # tensor_tensor, then the 1x-locked reduce sees half the
                # columns.
                hm = hmpool.tile([H, WIDE // 2], bf16, tag="hm")
                hmv = hm[:, : d * hw].rearrange("p (d w) -> p d w", d=d)
                nc.vector.tensor_tensor(
                    hmv, h3v[:, :, :hw], h3v[:, :, hw:wt], op=amax
                )
                nc.vector.reduce_max(
                    maxP[:, k0 : k0 + d], hmv, axis=mybir.AxisListType.X
                )
                ht = htpool.tile([H, WIDE // 2], bf16, tag="ht")
                htv = ht[:, : d * hw].rearrange("p (d w) -> p d w", d=d)
                nc.vector.tensor_tensor(
                    htv, h3v[:, :, :hw], h3v[:, :, hw:wt], op=add
                )
                nc.vector.reduce_sum(
                    sumP[:, k0 : k0 + d], htv, axis=mybir.AxisListType.X
                )

            prev = None
            for ti, (k0, d, wt, col0) in enumerate(tiles):
                tcols = d * wt
                xt = xpool.tile([C, WIDE], bf16, tag="xt")
                nc.sync.dma_start(xt[:, :tcols], xT[:, col0 : col0 + tcols])

                s0 = min(tcols, 512)
                split = R1B != 512
                b1x = max(min(R1B, s0), tcols - 512) if tcols > 512 else tcols
                if split:
                    # two single-bank p1 tiles: the ACT/DVE boundary can sit
                    # anywhere in [tcols-512, 512] without bank-crossing APs
                    p1aw = ps1.tile([H, 512], f32, tag="p1a")
                    p1a = p1aw[:, :b1x]
                    xa = xt[:, :b1x]
                    if tcols > b1x:
                        p1bw = ps1.tile([H, 512], f32, tag="p1b")
                        p1b = p1bw[:, : tcols - b1x]
                        xb = xt[:, b1x:tcols]
                    else:
                        p1b = None
                        xb = None
                else:
                    p1 = ps1.tile([H, WIDE], f32, tag="p1")
                    p1a = p1[:, :s0]
                    p1b = p1[:, 512:tcols] if tcols > 512 else None
                    xa = xt[:, :s0]
                    xb = xt[:, 512:tcols] if tcols > 512 else None
                nc.tensor.matmul(p1a, w1s[:], xa)
                if p1b is not None:
                    nc.tensor.matmul(p1b, w1s[:], xb)
                for _ in range(LDW_PAD):
                    nc.tensor.ldweights(w1s[:, :4])
                if prev is not None:
                    consume_p3_relu(*prev)
                h1 = h1pool.tile([H, WIDE], bf16, tag="h1")
                # relu1 halves run on ACT and DVE in parallel so p1 (the
                # single-buffered stage gating mm1 of the next tile) frees
                # in one half-pass latency.
                nc.scalar.activation(
                    h1[:, :b1x] if split else h1[:, :s0],
                    p1a, relu, bias=b1s[:],
                )
                if p1b is not None:
                    nc.vector.tensor_scalar(
                        h1[:, b1x:tcols] if split else h1[:, 512:tcols],
                        p1b, b1s[:], 0.0,
                        op0=add, op1=amax,
                    )

                if P2SPLIT:
                    p2aw = ps2.tile([H, 512], f32, tag="p2a")
                    p2a = p2aw[:, :s0]
                    p2b = None
                    nc.tensor.matmul(p2a, w2s[:], h1[:, :s0])
                    if tcols > 512:
                        p2bw = ps2.tile([H, 512], f32, tag="p2b")
                        p2b = p2bw[:, : tcols - 512]
                        nc.tensor.matmul(p2b, w2s[:], h1[:, 512:tcols])
                    for _ in range(LDW_PAD):
                        nc.tensor.ldweights(w1s[:, :4])
                    h2 = h2pool.tile([H, WIDE], bf16, tag="h2")
                    nc.scalar.activation(h2[:, :s0], p2a, relu, bias=b2s[:])
                    if p2b is not None:
                        nc.scalar.activation(
                            h2[:, 512:tcols], p2b, relu, bias=b2s[:]
                        )
                else:
                    p2 = ps2.tile([H, WIDE], f32, tag="p2")
                    nc.tensor.matmul(p2[:, :s0], w2s[:], h1[:, :s0])
                    if tcols > 512:
                        nc.tensor.matmul(p2[:, 512:tcols], w2s[:], h1[:, 512:tcols])
                    for _ in range(LDW_PAD):
                        nc.tensor.ldweights(w1s[:, :4])
                    h2 = h2pool.tile([H, WIDE], bf16, tag="h2")
                    nc.scalar.activation(h2[:, :tcols], p2[:, :tcols], relu, bias=b2s[:])

                p3 = ps3.tile([H, WIDE], f32, tag="p3")
                nc.tensor.matmul(p3[:, :s0], w3s[:], h2[:, :s0])
                if tcols > 512:
                    nc.tensor.matmul(p3[:, 512:tcols], w3s[:], h2[:, 512:tcols])

                # Deferred consumption of the PREVIOUS tile's p3 keeps the
                # ACT queue from head-of-line-blocking on the 3-matmul chain.
                if prev is not None:
                    consume_p3_reduce(*prev)
                h3 = h3pool.tile([H, WIDE], bf16, tag="h3")
                prev = (p3, h3, k0, d, wt, tcols)
            consume_p3_relu(*prev)
            consume_p3_reduce(*prev)

            # ---- epilogue: out[k, :] = sum_k @ Wsum + max_k @ Wmax
            #                + (sum_k * recip_k) @ Wmean + bo ----
            for ch in range(S // H):  # 2 chunks of 128 segments
                sl = slice(ch * H, (ch + 1) * H)
                if R1B != 512:
                    pow_ = ps1.tile([H, 512], f32, tag="p1a")
                else:
                    pow_ = ps1.tile([H, WIDE], f32, tag="p1")
                po = pow_[:, :O]
                nc.tensor.matmul(po[:], sumP[:, sl], wsums[:], start=True, stop=False)
                nc.tensor.matmul(po[:], maxP[:, sl], wmaxs[:], start=False, stop=False)
                nc.tensor.matmul(po[:], ones[:], bos[:], start=False, stop=True)

                if P2SPLIT:
                    pmw = ps2.tile([H, 512], f32, tag="p2a")
                else:
                    pmw = ps2.tile([H, WIDE], f32, tag="p2")
                pm = pmw[:, :O]
                nc.tensor.matmul(pm[:], sumP[:, sl], wmeans[:], start=True, stop=True)

                om = h1pool.tile([H, O], f32, tag="om")
                nc.vector.tensor_scalar_mul(om[:], pm[:], recs[:, ch : ch + 1])
                ot = h2pool.tile([H, O], f32, tag="ot")
                nc.vector.tensor_tensor(ot[:], po[:], om[:], op=add)
                nc.sync.dma_start(out[sl, :], ot[:])

    nc.compile()
    return nc


def kernel(**inputs):
    x = np.asarray(inputs["x"], dtype=np.float32)
    batch = np.asarray(inputs["batch"]).astype(np.int64)

    # ---- fold BN into the linears ----
    W1p, b1p = _fold_bn(
        np.asarray(inputs["W1"]), np.asarray(inputs["b1"]),
        np.asarray(inputs["g1"]), np.asarray(inputs["be1"]),
        np.asarray(inputs["m1"]), np.asarray(inputs["v1"]),
    )
    W2p, b2p = _fold_bn(
        np.asarray(inputs["W2"]), np.asarray(inputs["b2"]),
        np.asarray(inputs["g2"]), np.asarray(inputs["be2"]),
        np.asarray(inputs["m2"]), np.asarray(inputs["v2"]),
    )
    W3p, b3p = _fold_bn(
        np.asarray(inputs["W3"]), np.asarray(inputs["b3"]),
        np.asarray(inputs["g3"]), np.asarray(inputs["be3"]),
        np.asarray(inputs["m3"]), np.asarray(inputs["v3"]),
    )
    Wop, bop = _fold_bn(
        np.asarray(inputs["Wo"]), np.asarray(inputs["bo"]),
        np.asarray(inputs["go"]), np.asarray(inputs["beo"]),
        np.asarray(inputs["mo"]), np.asarray(inputs["vo"]),
    )

    # ---- whole-segment sharding by sorted-width round-robin rank ----
    counts = np.bincount(batch, minlength=NSEG).astype(np.int64)
    assert np.all(batch[:-1] <= batch[1:]), "batch must be sorted"
    order = np.argsort(-counts, kind="stable")  # segment ids, width desc
    slot_w = np.maximum(counts[order[::NCORES][:S]], 1)  # width of rank 8k
    tiles, cols = _plan_tiles(slot_w)

    key = (cols, tuple(slot_w.tolist()), LDW_PAD, R1B, P2SPLIT)
    if key not in _compiled_cache:
        _compiled_cache[key] = _build_program(tiles, cols)
    nc = _compiled_cache[key]

    # column start and padded width of each slot
    slot_col = np.zeros(S, dtype=np.int64)
    slot_wt = np.zeros(S, dtype=np.int64)
    for k0, d, wt, col0 in tiles:
        for j in range(d):
            slot_col[k0 + j] = col0 + j * wt
            slot_wt[k0 + j] = wt

    starts = np.searchsorted(batch, np.arange(NSEG), side="left")

    bf = mybir.dt.np(mybir.dt.bfloat16)
    W1bf = W1p.astype(bf).astype(np.float32)
    W2bf = W2p.astype(bf).astype(np.float32)
    W3bf = W3p.astype(bf).astype(np.float32)

    in_maps = []
    core_segs = []
    for c in range(NCORES):
        segs = order[np.arange(S) * NCORES + c]  # this core's segment ids
        core_segs.append(segs)
        src = np.zeros(cols, dtype=np.int64)
        emptyc = np.zeros(cols, dtype=bool)
        for k in range(S):
            s = segs[k]
            cnt = int(counts[s])
            c0 = slot_col[k]
            wt = int(slot_wt[k])
            if cnt:
                src[c0 : c0 + cnt] = np.arange(starts[s], starts[s] + cnt)
                # dup-pad with the segment's first node
                src[c0 + cnt : c0 + wt] = starts[s]
            else:
                emptyc[c0 : c0 + wt] = True
        xTc = x[src].T.astype(bf)
        if emptyc.any():
            xTc[:, emptyc] = 0
        recipc = (1.0 / np.maximum(counts[segs], 1.0)).astype(np.float32)
        in_maps.append(
            dict(
                xT=np.ascontiguousarray(xTc),
                w1=np.ascontiguousarray(W1p.T).astype(bf),
                w2=np.ascontiguousarray(W2p.T).astype(bf),
                w3=np.ascontiguousarray(W3p.T).astype(bf),
                b1=np.ascontiguousarray(b1p[:, None]),
                b2=np.ascontiguousarray(b2p[:, None]),
                b3=np.ascontiguousarray(b3p[:, None]),
                wsum=np.ascontiguousarray(Wop[:, 0:H].T),
                wmax=np.ascontiguousarray(Wop[:, H : 2 * H].T),
                wmean=np.ascontiguousarray(Wop[:, 2 * H : 3 * H].T),
                bo=np.ascontiguousarray(bop[None, :]),
                recip=np.ascontiguousarray(recipc.reshape(S // H, H).T),
            )
        )

    ncores_run = int(os.environ.get("KERNEL_NCORES", str(NCORES)))
    res = bass_utils.run_bass_kernel_spmd(
        nc,
        in_maps[:ncores_run],
        core_ids=list(range(ncores_run)),
        trace=bool(int(os.environ.get("KERNEL_TRACE", "0"))),
        tmpdir=os.environ.get("KERNEL_TRACE_DIR") or None,
    )
    kernel.last_results = res

    # ---- host-side dup-pad correction ----
    # Replay the bf16 encoder for every segment's first node, matching the
    # device values (bf16 weights/activations, fp32 accumulate) to ~1ulp.
    first = x[starts[order[: S * NCORES]].clip(0)]  # [S*NCORES, C] rank order
    xf = first.astype(bf).astype(np.float32)
    h1f = np.maximum(xf @ W1bf.T + b1p, 0.0).astype(bf).astype(np.float32)
    h2f = np.maximum(h1f @ W2bf.T + b2p, 0.0).astype(bf).astype(np.float32)
    h3f = np.maximum(h2f @ W3bf.T + b3p, 0.0).astype(bf).astype(np.float32)

    out_full = np.zeros((NSEG, O), dtype=np.float32)
    ranks = np.arange(S)
    for c in range(ncores_run):
        segs = core_segs[c]
        o = np.array(res.results[c]["out"], dtype=np.float32)
        npads = (slot_wt - counts[segs]).astype(np.float32)
        h3c = h3f[ranks * NCORES + c]  # [S, H] first-node h3 per slot
        recipc = 1.0 / np.maximum(counts[segs], 1.0)
        corr = (h3c @ Wop[:, 0:H].T) * npads[:, None]
        corr += (h3c @ Wop[:, 2 * H : 3 * H].T) * (npads * recipc)[:, None]
        o -= corr
        empty = counts[segs] == 0
        if empty.any():
            o[empty] = bop[None, :]
        out_full[segs] = o
    return out_full

